# revision 1
# baseline (speedup 1.0000x reference)
"""Trainium2 Bass kernel for nn_BottomUpIntegrator (gnn_message_passing).

Sharding: cells split at cluster boundaries across 8 cores (2048 clusters
each). Per-core segmented reductions via one-hot scatter matmuls accumulating
into 2 rotating PSUM banks (one per active 512-cluster block) with a
core-invariant static window schedule (W=8).

Pipeline (per 8192-cell chunk, software-pipelined across 3 iterations):
  DMA: feats fp8(e3m4) [73,4096] (bias folded as ones-row), host-packed
       bf16 [1|a|sur|a^2|eph] columns + one-hot window rows, one merged DMA.
  PE:  mm1 (h = W1^T f), mm2 (base logits, hs-as-weights trick), scatter
       (vt12 host rows -> PSUM parts 32:42, ew rows -> parts 0:5).
  ACT: relu x4 (tiles q0,2,4,6), tanh (sigmoid = .5+.5 tanh(v/2); Tanh/Exp/
       Relu/Copy share one activation table - no table thrash), exp(w).
  DVE: relu x4 (tiles q1,3,5,7), block drains.
  Pool (SBUF-only): sigmoid affine, clip chain, ew*[1,a] scatter rows.
Cluster phase: strided transposes put cluster 16p+b on partition p (contiguous
output DMA), stats chain + cluster MLP with clusters on partitions (V1 bias
via ones-feature, V2 dot via elementwise+reduce; no transposes back).
Organism-level finale (valid mask, argmax/present, softmax weights) runs on
host in f64 from per-cluster outputs + impc (host knows exact counts).
"""
import numpy as np
import ml_dtypes

import json as _json

from concourse import bass, mybir
from concourse import bass2jax as _b2j
from concourse import bass_utils as _bu
from concourse.tile import TileContext
from concourse.bass_utils import run_bass_kernel_spmd

_orig_compile = _bu.compile_bir_kernel


def _split_waits_compile(bir_json, tmpdir, neff_name="file.neff"):
    """Walrus lowers at most ONE semaphore wait per TPB instruction struct.
    Tile emits several. Hoist extras onto injected same-engine EventSemaphore
    wait instructions immediately before the owner (semantically identical:
    engines execute in program order)."""
    d = _json.loads(bir_json)
    cnt = 0
    for fn in d["functions"]:
        for blk in fn["blocks"]:
            newlist = []
            for ins in blk["instructions"]:
                si = ins.get("sync_info")
                waits = si.get("on_wait", []) if si else []
                if si and len(waits) > 1 and ins.get("opcode") not in (
                        "EventSemaphore",):
                    for w_i, w in enumerate(waits[:-1]):
                        cnt += 1
                        newlist.append({
                            "debug": ins.get("debug", 0),
                            "engine": ins["engine"],
                            "ins": [], "outs": [],
                            "name": f"{ins['name']}-wsplit{w_i}",
                            "opcode": "EventSemaphore",
                            "sync_info": {"on_update": [], "on_wait": [w]},
                        })
                    si["on_wait"] = [waits[-1]]
                newlist.append(ins)
            blk["instructions"] = newlist
    print(f"[wait-split] hoisted {cnt} extra waits")
    return _orig_compile(_json.dumps(d).encode(), tmpdir, neff_name=neff_name)


_bu.compile_bir_kernel = _split_waits_compile
_b2j.compile_bir_kernel = _split_waits_compile

F32 = mybir.dt.float32
BF16 = mybir.dt.bfloat16
FP8 = mybir.dt.float8e3
AF = mybir.ActivationFunctionType
OP = mybir.AluOpType
AX = mybir.AxisListType

NCORES = 8
KLOC = 2048            # clusters per core
NPAD = 262144          # padded cells per core
CHUNK = 8192           # cells per chunk
NCHUNK = NPAD // CHUNK # 32
W = 8                  # onehot window width (clusters)
MARGIN = 2             # window start = clip(s - MARGIN, 0, 512 - W)
NTILES = NPAD // 128   # 2048 scatter tiles per core
TPB = NTILES // 4      # tiles per 512-cluster block
PADSEG = 1.0e9

# hs is bf16 throughout; relu per 512-col mm1 tile, ACT on even tiles and
# DVE on odd ones (Pool cannot touch PSUM, so only ACT/DVE drain mm1 banks)
HSB_COLS = 4096

# scatter stationary rows, accumulated in scat banks [48, 512]:
#  parts 0:5   <- vt5  (device): [ew, ew*a4]
#  parts 32:42 <- vt12 (host):   [1, a4, sur, a2_4]  (lhsT cols 0:10)
# vt12 (host) col layout: [1, a4, sur, a2_4, eph, pad]


def _window_start(S):
    s = S % TPB
    return int(np.clip(s - MARGIN, 0, 512 - W))


def build_program():
    nc = bass.Bass(trn_type="TRN2", use_seq_codegen=True)
    featsT = nc.dram_tensor("featsT", [73, NPAD // 2], FP8, kind="ExternalInput")
    vohd = nc.dram_tensor("vohd", [NCHUNK, 128, 768 + 64 * W], BF16,
                          kind="ExternalInput")
    w1d = nc.dram_tensor("w1d", [73, 128], FP8, kind="ExternalInput")
    cbfd = nc.dram_tensor("cbfd", [128, 674], BF16, kind="ExternalInput")
    cf32d = nc.dram_tensor("cf32d", [128, 130], F32, kind="ExternalInput")
    out_all = nc.dram_tensor("out_all", [128, 144], F32, kind="ExternalOutput")

    with TileContext(nc) as tc:
        with (
            tc.tile_pool(name="const", bufs=1) as cp,
            tc.tile_pool(name="feats", bufs=3) as fp,
            tc.tile_pool(name="hsp", bufs=3) as hp,
            tc.tile_pool(name="small", bufs=3) as sp,
            tc.tile_pool(name="scatv", bufs=4) as vp,
            tc.tile_pool(name="ph_b", bufs=1) as bp,
        ):
            # ---- constants ----------------------------------------------
            w1s = cp.tile([73, 128], FP8, tag="w1s")
            nc.sync.dma_start(out=w1s[:], in_=w1d[:])
            cbf = cp.tile([128, 674], BF16, tag="cbf")
            nc.sync.dma_start(out=cbf[:], in_=cbfd[:])
            cf32 = cp.tile([128, 130], F32, tag="cf32")
            nc.sync.dma_start(out=cf32[:], in_=cf32d[:])
            w2bs = cbf[:, 0:2]
            v2rep = cbf[:, 2:514]
            v1s8 = cbf[0:8, 514:546]
            idb_c = cbf[:, 546:674]
            b2hs = cf32[:, 0:1]
            c2h = cf32[:, 1:2]
            ids = cf32[:, 2:130]
            zbf = cp.tile([128, 512], BF16, tag="zbf")
            nc.vector.memset(zbf[:], 0.0)

            # Pre-touch DMA-loaded constants on their consuming engines so no
            # later compute instruction needs a second (DMA) semaphore wait.
            scra = cp.tile([128, 2], F32, tag="scra")
            nc.scalar.activation(out=scra[:, 0:1], in_=b2hs, func=AF.Copy)
            nc.scalar.activation(out=scra[:, 1:2], in_=c2h, func=AF.Copy)

            # persistent scatter accumulators: 2 rotating PSUM banks [48, 512]
            # dev rows (ew, ewa4) at partitions 0:5, host rows (1, a4, sur,
            # a2_4) at partitions 32:42; zero-initialized by all-zero matmuls.
            pps_cm = tc.tile_pool(name="scatps", bufs=1, space="PSUM")
            pps = pps_cm.__enter__()
            scat = [pps.tile([48, 512], F32, tag=f"scat{b}", name=f"scat{b}")
                    for b in range(2)]
            # PE touch of PE-consumed consts (rides on Ldweights; overwritten
            # by the zeroing matmul below).
            nc.tensor.matmul(out=scat[0][0:1, 0:1], lhsT=ids[0:1, 0:1],
                             rhs=ids[0:1, 0:1], start=True, stop=True,
                             skip_group_check=True)
            nc.tensor.matmul(out=scat[0][0:1, 0:2], lhsT=w1s[0:1, 0:1],
                             rhs=w1s[0:1, 0:2], start=True, stop=True,
                             skip_group_check=True)
            nc.tensor.matmul(out=scat[0][0:2, 0:2], lhsT=w2bs[0:1, :],
                             rhs=w2bs[0:1, :], start=True, stop=True,
                             skip_group_check=True)
            for b in range(2):
                nc.tensor.matmul(out=scat[b][:], lhsT=zbf[:, 0:48], rhs=zbf[:],
                                 start=True, stop=False, skip_group_check=True)
            sc = bp.tile([48, 2048], F32, tag="sc")

            # ---- phase A ------------------------------------------------
            with (
                tc.tile_pool(name="mm1ps", bufs=5, space="PSUM") as pp1,
                tc.tile_pool(name="mm2ps", bufs=1, space="PSUM") as pp2,
            ):
                # software pipeline: at iteration it --
                #   DMA(it) | mm1+relu(it-1) | mm2+chain(it-2) | scatter(it-3)
                T = {}

                def dma_stage(k):
                    ft = fp.tile([73, 4096], FP8, tag="ft")
                    nc.sync.dma_start(out=ft[:],
                                      in_=featsT[:, k * 4096:(k + 1) * 4096])
                    voh = vp.tile([128, 768 + 64 * W], BF16, tag="voh")
                    nc.sync.dma_start(out=voh[:], in_=vohd[k])
                    T[k] = dict(ft=ft, vt12=voh[:, 0:768], oh=voh[:, 768:])

                def mm1_stage(k, half):
                    # half 0: 512-col tiles q0..q3; half 1: q4..q7; relu per
                    # tile alternating ACT/DVE (Pool cannot touch PSUM)
                    d = T[k]
                    if half == 0:
                        d["hsb"] = hp.tile([128, HSB_COLS], BF16, tag="hsb", name=f"hsb{k}")
                    for q in range(4 * half, 4 * half + 4):
                        hp1 = pp1.tile([128, 512], F32, tag="hp1")
                        nc.tensor.matmul(
                            out=hp1[:], lhsT=w1s[:],
                            rhs=d["ft"][:, 512 * q:512 * (q + 1)],
                            start=True, stop=True)
                        dst_ap = d["hsb"][:, 512 * q:512 * (q + 1)]
                        if q % 2 == 0:
                            nc.scalar.activation(out=dst_ap, in_=hp1[:],
                                                 func=AF.Relu)
                        else:
                            nc.vector.tensor_scalar(out=dst_ap, in0=hp1[:],
                                                    scalar1=0.0, scalar2=None,
                                                    op0=OP.max)

                def mm2_stage(k):
                    d = T[k]
                    hsb = d["hsb"]
                    bb = pp2.tile([128, 64], F32, tag="bb")
                    d["bb"] = bb
                    for tt in range(32):
                        nc.tensor.matmul(
                            out=bb[:, 2 * tt:2 * tt + 2],
                            lhsT=hsb[:, 128 * tt:128 * (tt + 1)],
                            rhs=w2bs[:],
                            start=(tt == 0), stop=(tt == 31),
                            skip_group_check=True)

                def chain_a(k):
                    # sig(v) = .5 + .5*tanh(.5*v); x = clip(sig*eph, .01, 1)
                    # w = x*eph; ew = exp(w); vt16 rows 0:10 host, 10:15 dev
                    d = T[k]
                    vt12v = d["vt12"].rearrange("p (s v) -> p s v", v=12)
                    th = sp.tile([128, 64], F32, tag="th")
                    nc.scalar.activation(out=th[:], in_=d["bb"][:], func=AF.Tanh,
                                         bias=b2hs, scale=0.5)
                    sg = sp.tile([128, 64], F32, tag="sg")
                    nc.gpsimd.tensor_scalar(out=sg[:], in0=th[:], scalar1=0.5,
                                            scalar2=0.5, op0=OP.mult, op1=OP.add)
                    xw = sp.tile([128, 128], F32, tag="xw")
                    xv = xw[:, 0:64]
                    wv = xw[:, 64:128]
                    nc.gpsimd.tensor_tensor(
                        out=xv.rearrange("p (s o) -> p s o", o=1),
                        in0=sg[:].rearrange("p (s o) -> p s o", o=1),
                        in1=vt12v[:, :, 10:11], op=OP.mult)
                    nc.gpsimd.tensor_scalar(out=xv, in0=xv, scalar1=0.01,
                                            scalar2=1.0, op0=OP.max, op1=OP.min)
                    nc.gpsimd.tensor_tensor(
                        out=wv.rearrange("p (s o) -> p s o", o=1),
                        in0=xv.rearrange("p (s o) -> p s o", o=1),
                        in1=vt12v[:, :, 10:11], op=OP.mult)
                    d["wv"] = wv

                def chain_b(k):
                    d = T[k]
                    vt12v = d["vt12"].rearrange("p (s v) -> p s v", v=12)
                    ewt = sp.tile([128, 64], BF16, tag="ewt")
                    nc.scalar.activation(out=ewt[:], in_=d["wv"], func=AF.Exp)
                    vt5 = vp.tile([128, 5 * 64], BF16, tag="vt5")
                    v5 = vt5[:].rearrange("p (s v) -> p s v", v=5)
                    nc.gpsimd.tensor_tensor(out=v5,
                                            in0=vt12v[:, :, 0:5],
                                            in1=ewt[:].to_broadcast([128, 64, 5]),
                                            op=OP.mult)
                    d["vt5"] = vt5

                def scatter_stage(k):
                    # col j -> sorted tile S = 64k + 32*(j%2) + j//2
                    d = T[k]
                    sbank = scat[(k // 8) % 2]
                    vt12 = d["vt12"]
                    vt5 = d["vt5"]
                    for j in range(64):
                        S = 64 * k + 32 * (j % 2) + (j // 2)
                        f = _window_start(S)
                        last = (k % 8 == 7 and j >= 62)
                        nc.tensor.matmul(out=sbank[0:5, f:f + W],
                                         lhsT=vt5[:, 5 * j:5 * j + 5],
                                         rhs=d["oh"][:, W * j:W * j + W],
                                         start=False, stop=last,
                                         skip_group_check=True)
                        nc.tensor.matmul(out=sbank[32:42, f:f + W],
                                         lhsT=vt12[:, 12 * j:12 * j + 10],
                                         rhs=d["oh"][:, W * j:W * j + W],
                                         start=False, stop=last,
                                         skip_group_check=True)
                    if k % 8 == 7:
                        blk = k // 8
                        nc.vector.tensor_copy(
                            out=sc[:, 512 * blk:512 * (blk + 1)], in_=sbank[:])
                        if blk < 2:
                            nc.tensor.matmul(out=sbank[:], lhsT=zbf[:, 0:48],
                                             rhs=zbf[:], start=True, stop=False,
                                             skip_group_check=True)
                    # free stale per-chunk state
                    del T[k]

                for it in range(NCHUNK + 3):
                    if it < NCHUNK:
                        dma_stage(it)
                    if 1 <= it <= NCHUNK:
                        mm1_stage(it - 1, 0)
                    if 2 <= it <= NCHUNK + 1:
                        mm2_stage(it - 2)
                        chain_a(it - 2)
                    if 3 <= it <= NCHUNK + 2:
                        scatter_stage(it - 3)
                    if 1 <= it <= NCHUNK:
                        mm1_stage(it - 1, 1)
                    if 2 <= it <= NCHUNK + 1:
                        chain_b(it - 2)

            # ---- phase B ------------------------------------------------
            # sc rows: 0=sew, 1:5=sewa, 32=cnt, 33:37=sa, 37=ssur, 38:42=ssq
            tc.strict_bb_all_engine_barrier()
            pps_cm.__exit__(None, None, None)

            with (
                tc.tile_pool(name="ptps", bufs=4, space="PSUM") as ppt,
                tc.tile_pool(name="ptcs", bufs=3, space="PSUM") as pptc,
                tc.tile_pool(name="mmbps", bufs=1, space="PSUM") as ppm,
            ):
                tt = bp.tile([128, 16 * 48], F32, tag="tt")
                scv = sc[:].rearrange("p (c g) -> p g c", g=16)
                for b in range(16):
                    pt = ppt.tile([128, 48], F32, tag="pt")
                    nc.tensor.transpose(out=pt[:], in_=scv[:, b, :],
                                        identity=ids[0:48, 0:48])
                    if b % 2 == 0:
                        nc.vector.tensor_copy(out=tt[:, 48 * b:48 * (b + 1)],
                                              in_=pt[:])
                    else:
                        nc.scalar.activation(out=tt[:, 48 * b:48 * (b + 1)],
                                             in_=pt[:], func=AF.Copy)
                tv = tt[:].rearrange("p (b q) -> p b q", q=48)
                cnt = tv[:, :, 32:33]    # [128,16,1]
                sa = tv[:, :, 33:37]
                ssur = tv[:, :, 37:38]
                ssq = tv[:, :, 38:42]
                sew = tv[:, :, 0:1]
                sewa = tv[:, :, 1:5]

                def wt(tag):
                    return bp.tile([128, 16], F32, tag=tag, name=tag)

                def v3(t):
                    return t[:].rearrange("p (b a) -> p b a", a=1)

                def w4(tag):
                    t = bp.tile([128, 64], F32, tag=tag, name=tag)
                    return t, t[:].rearrange("p (b a) -> p b a", a=4)

                cntc = wt("cntc")
                nc.vector.tensor_scalar(out=v3(cntc), in0=cnt, scalar1=1.0,
                                        scalar2=None, op0=OP.max)
                rc = wt("rc")
                nc.vector.reciprocal(out=rc[:], in_=cntc[:])
                den = wt("den")
                nc.vector.tensor_scalar(out=v3(den), in0=sew, scalar1=1.0,
                                        scalar2=None, op0=OP.max)
                rden = wt("rden")
                nc.vector.reciprocal(out=rden[:], in_=den[:])
                agr, agrv = w4("agr")
                nc.vector.tensor_tensor(out=agrv, in0=sewa,
                                        in1=rden[:].to_broadcast([128, 16, 4]),
                                        op=OP.mult)
                es, esv = w4("es")
                nc.scalar.activation(out=es[:], in_=agr[:], func=AF.Exp)
                ssum = wt("ssum")
                nc.vector.tensor_reduce(out=v3(ssum), in_=esv, axis=AX.X, op=OP.add)
                rssum = wt("rssum")
                nc.vector.reciprocal(out=rssum[:], in_=ssum[:])
                agg, aggv = w4("agg")
                nc.vector.tensor_tensor(out=aggv, in0=esv,
                                        in1=rssum[:].to_broadcast([128, 16, 4]),
                                        op=OP.mult)
                mean, meanv = w4("mean")
                nc.vector.tensor_tensor(out=meanv, in0=sa,
                                        in1=rc[:].to_broadcast([128, 16, 4]),
                                        op=OP.mult)
                var, varv = w4("var")
                nc.vector.tensor_tensor(out=varv, in0=meanv, in1=meanv, op=OP.mult)
                cntb = wt("cntb")
                nc.vector.tensor_copy(out=v3(cntb), in_=cnt)
                nc.vector.tensor_tensor(out=varv, in0=varv,
                                        in1=cntb[:].to_broadcast([128, 16, 4]),
                                        op=OP.mult)
                nc.vector.tensor_tensor(out=varv, in0=ssq, in1=varv,
                                        op=OP.subtract)
                cm1 = wt("cm1")
                nc.vector.tensor_scalar(out=v3(cm1), in0=cnt, scalar1=-1.0,
                                        scalar2=1.0, op0=OP.add, op1=OP.max)
                rcm1 = wt("rcm1")
                nc.vector.reciprocal(out=rcm1[:], in_=cm1[:])
                nc.vector.tensor_tensor(out=varv, in0=varv,
                                        in1=rcm1[:].to_broadcast([128, 16, 4]),
                                        op=OP.mult)
                vm = wt("vm")
                nc.vector.tensor_reduce(out=v3(vm), in_=varv, axis=AX.X, op=OP.add)
                nc.vector.tensor_scalar(out=vm[:], in0=vm[:], scalar1=0.25,
                                        scalar2=None, op0=OP.mult)
                phic = wt("phic")
                nc.vector.tensor_scalar(out=phic[:], in0=vm[:], scalar1=2.0,
                                        scalar2=1.0, op0=OP.mult, op1=OP.min)
                nc.vector.tensor_scalar(out=phic[:], in0=phic[:], scalar1=-1.0,
                                        scalar2=1.0, op0=OP.mult, op1=OP.add)
                coh = wt("coh")
                nc.vector.tensor_scalar(out=coh[:], in0=vm[:], scalar1=-1.0,
                                        scalar2=1.0, op0=OP.mult, op1=OP.add)
                perr = wt("perr")
                nc.vector.tensor_tensor(out=v3(perr), in0=ssur, in1=v3(rc),
                                        op=OP.mult)
                integ = wt("integ")
                nc.vector.tensor_scalar(out=integ[:], in0=perr[:], scalar1=-1.0,
                                        scalar2=1.0, op0=OP.mult, op1=OP.add)
                nc.vector.tensor_tensor(out=integ[:], in0=integ[:], in1=phic[:],
                                        op=OP.mult)

                # cluster MLP, clusters on partitions throughout:
                # hc = relu(cftt.T @ v1s8) per 128-cluster block, then
                # base = tanh(.5*(hc . v2) + .5*c2) -> sig affine
                cft = bp.tile([128, 16 * 8], BF16, tag="cft")
                cfv = cft[:].rearrange("p (b q) -> p b q", q=8)
                nc.vector.tensor_copy(out=cfv[:, :, 0:4], in_=aggv)
                nc.vector.tensor_copy(out=cfv[:, :, 4:5],
                                      in_=phic[:].to_broadcast([128, 16, 1]))
                nc.vector.tensor_copy(out=cfv[:, :, 5:6],
                                      in_=coh[:].to_broadcast([128, 16, 1]))
                szf = wt("szf")
                nc.vector.tensor_scalar(out=v3(szf), in0=cnt, scalar1=0.05,
                                        scalar2=1.0, op0=OP.mult, op1=OP.min)
                nc.vector.tensor_copy(out=cfv[:, :, 6:7],
                                      in_=szf[:].to_broadcast([128, 16, 1]))
                nc.vector.memset(cfv[:, :, 7:8], 1.0)
                cftt = bp.tile([8, 2048], BF16, tag="cftt")
                for b in range(16):
                    ptc = pptc.tile([128, 128], BF16, tag="ptc")
                    nc.tensor.transpose(out=ptc[0:8, :],
                                        in_=cft[:, 8 * b:8 * (b + 1)],
                                        identity=idb_c)
                    if b % 2 == 0:
                        nc.vector.tensor_copy(out=cftt[:, 128 * b:128 * (b + 1)],
                                              in_=ptc[0:8, :])
                    else:
                        nc.scalar.activation(out=cftt[:, 128 * b:128 * (b + 1)],
                                             in_=ptc[0:8, :], func=AF.Copy)
                hcp = ppm.tile([128, 512], F32, tag="hcp")
                for b in range(16):
                    nc.tensor.matmul(out=hcp[:, 32 * b:32 * (b + 1)],
                                     lhsT=cftt[:, 128 * b:128 * (b + 1)],
                                     rhs=v1s8, start=True, stop=True,
                                     skip_group_check=True)
                hcsb = bp.tile([128, 512], BF16, tag="hcsb")
                nc.scalar.activation(out=hcsb[:], in_=hcp[:], func=AF.Relu)
                hv2 = bp.tile([128, 512], F32, tag="hv2")
                nc.vector.tensor_tensor(out=hv2[:], in0=hcsb[:], in1=v2rep,
                                        op=OP.mult)
                bb2 = wt("bb2")
                nc.vector.tensor_reduce(
                    out=v3(bb2),
                    in_=hv2[:].rearrange("p (b h) -> p b h", h=32),
                    axis=AX.X, op=OP.add)
                basec = wt("basec")
                nc.scalar.activation(out=basec[:], in_=bb2[:], func=AF.Tanh,
                                     bias=c2h, scale=0.5)
                nc.vector.tensor_scalar(out=basec[:], in0=basec[:], scalar1=0.5,
                                        scalar2=0.5, op0=OP.mult, op1=OP.add)

                # cluster_out [2048, 8] + impc, one [128, 144] output tile
                oc = bp.tile([128, 144], F32, tag="oc")
                ocv = oc[:, 0:128].rearrange("p (b q) -> p b q", q=8)
                impc = oc[:, 128:144]
                nc.vector.tensor_tensor(out=impc, in0=basec[:], in1=phic[:],
                                        op=OP.mult)
                nc.vector.tensor_scalar(out=impc, in0=impc, scalar1=0.01,
                                        scalar2=1.0, op0=OP.max, op1=OP.min)
                nc.vector.tensor_copy(out=ocv[:, :, 0:4], in_=aggv)
                nc.vector.tensor_copy(out=ocv[:, :, 4:5],
                                      in_=phic[:].to_broadcast([128, 16, 1]))
                nc.vector.tensor_copy(out=ocv[:, :, 5:6],
                                      in_=coh[:].to_broadcast([128, 16, 1]))
                nc.vector.tensor_copy(out=ocv[:, :, 6:7],
                                      in_=perr[:].to_broadcast([128, 16, 1]))
                nc.vector.tensor_copy(out=ocv[:, :, 7:8],
                                      in_=integ[:].to_broadcast([128, 16, 1]))
                nc.sync.dma_start(out=out_all[:], in_=oc[:])
    return nc


_NC_CACHE = None


def _get_program():
    global _NC_CACHE
    if _NC_CACHE is None:
        _NC_CACHE = build_program()
    return _NC_CACHE


def _host_prep_core(c, state, arch, energy, phi_local, surprise, seg_ids):
    B0 = int(np.searchsorted(seg_ids, 2048 * c))
    B1 = int(np.searchsorted(seg_ids, 2048 * (c + 1)))
    Nc = B1 - B0
    lseg = (seg_ids[B0:B1] - 2048 * c).astype(np.int64)
    idx = np.full(NPAD, -1, np.int64)
    rel = np.full(NPAD, PADSEG, np.float32)
    cur = 0
    for S in range(NTILES):
        blk = S // TPB
        f = _window_start(S)
        wlo = 512 * blk + f
        whi = wlo + W
        take = min(128, int(np.searchsorted(lseg, whi)) - cur)
        if take > 0:
            assert lseg[cur] >= wlo, f"core {c} tile {S}: behind-lag"
            sl = np.arange(cur, cur + take)
            idx[S * 128:S * 128 + take] = sl
            rel[S * 128:S * 128 + take] = (lseg[sl] - wlo).astype(np.float32)
            cur += take
    assert cur == Nc, f"core {c}: {Nc - cur} cells not scheduled"
    m = idx >= 0

    def g(x):
        out = np.zeros((NPAD,) + x.shape[1:], np.float32)
        out[m] = x[B0:B1][idx[m]]
        return out

    return g(state), g(arch), g(energy), g(phi_local), g(surprise), rel, m


def _swz(x):
    """[NPAD, Q] cell-major -> [NCHUNK, 128, 64*Q] device layout."""
    Q = x.shape[1]
    return np.ascontiguousarray(
        x.reshape(NCHUNK, 2, 32, 128, Q).transpose(0, 3, 2, 1, 4).reshape(
            NCHUNK, 128, 64 * Q))


def kernel(state, arch, energy, phi_local, surprise, seg_ids, n_clusters,
           W1, b1, W2, b2, V1, c1, V2, c2):
    state = np.asarray(state, np.float32)
    arch = np.asarray(arch, np.float32)
    energy = np.asarray(energy, np.float32)
    phi_local = np.asarray(phi_local, np.float32)
    surprise = np.asarray(surprise, np.float32)
    seg_ids = np.asarray(seg_ids)
    W1 = np.asarray(W1, np.float32); b1 = np.asarray(b1, np.float32)
    W2 = np.asarray(W2, np.float32); b2 = np.asarray(b2, np.float32)
    V1 = np.asarray(V1, np.float32); c1 = np.asarray(c1, np.float32)
    V2 = np.asarray(V2, np.float32); c2 = np.asarray(c2, np.float32)

    w1d = np.zeros((73, 128), np.float32)
    w1d[0:36, 0:64] = W1
    w1d[36:72, 64:128] = W1
    w1d[72, 0:64] = b1
    w1d[72, 64:128] = b1
    w2f = np.zeros((128, 2), np.float32)
    w2f[0:64, 0] = W2[:, 0]
    w2f[64:128, 1] = W2[:, 0]
    cbf = np.zeros((128, 674), np.float32)
    cbf[:, 0:2] = w2f
    cbf[:, 2:514] = np.tile(V2[:, 0], (128, 16))
    cbf[0:8, 514:546] = np.concatenate([V1, c1.reshape(1, 32)], 0)
    cbf[:, 546:674] = np.eye(128, dtype=np.float32)
    cf32 = np.zeros((128, 130), np.float32)
    cf32[:, 0] = 0.5 * b2[0]
    cf32[:, 1] = 0.5 * c2[0]
    cf32[:, 2:130] = np.eye(128, dtype=np.float32)
    consts = dict(
        w1d=w1d.astype(ml_dtypes.float8_e3m4),
        cbfd=cbf.astype(ml_dtypes.bfloat16),
        cf32d=cf32,
    )
    iw = np.arange(W, dtype=np.float32)

    def _prep(c):
        st, ar, en, ph, su, rel, msk = _host_prep_core(
            c, state, arch, energy, phi_local, surprise, seg_ids)
        f36 = np.concatenate([st.T, ar.T], 0)              # [36, NPAD]
        featsT = np.concatenate(
            [f36.reshape(36, NCHUNK, 2, 4096).transpose(2, 0, 1, 3).reshape(
                72, NPAD // 2),
             np.ones((1, NPAD // 2), np.float32)], 0).astype(ml_dtypes.float8_e3m4)
        # vt12: [1(mask), a4, sur, a2_4, eph, pad]
        vt12 = np.zeros((NPAD, 12), np.float32)
        vt12[:, 0] = msk
        vt12[:, 1:5] = ar
        vt12[:, 5] = su
        vt12[:, 6:10] = ar * ar
        vt12[:, 10] = en * ph
        oh = (rel[:, None] == iw[None, :]).astype(np.float32)   # [NPAD, W]
        voh = np.concatenate([_swz(vt12), _swz(oh)], axis=2)
        return dict(featsT=np.ascontiguousarray(featsT),
                    vohd=np.ascontiguousarray(voh).astype(ml_dtypes.bfloat16),
                    **consts)

    from concurrent.futures import ThreadPoolExecutor
    with ThreadPoolExecutor(NCORES) as ex:
        in_maps = list(ex.map(_prep, range(NCORES)))
    nc = _get_program()
    res = run_bass_kernel_spmd(nc, in_maps, list(range(NCORES)))
    global LAST_RESULT
    LAST_RESULT = res
    outs = res.results
    alls = [np.asarray(outs[c]["out_all"]) for c in range(NCORES)]
    couts = [a[:, 0:128].reshape(2048, 8) for a in alls]
    impcs = [a[:, 128:144].reshape(-1) for a in alls]
    cluster_full = np.concatenate(couts, 0).astype(np.float32)
    impc = np.concatenate(impcs, 0).astype(np.float64)

    # organism-level finale on host (exact, f64)
    K = 16384
    counts = np.bincount(seg_ids, minlength=K)
    valid = counts > 0
    n_valid = max(float(valid.sum()), 1.0)
    aggregate = cluster_full[:, 0:4].astype(np.float64)
    phi_c = cluster_full[:, 4].astype(np.float64)
    coh = cluster_full[:, 5].astype(np.float64)
    iv = np.where(valid, impc, -np.inf)
    e = np.exp(iv - iv.max())
    wc = e / e.sum()
    ga = (wc[:, None] * aggregate).sum(0)
    eg = np.exp(ga - ga.max())
    global_arch = (eg / eg.sum()).astype(np.float32)
    avg_phi = (phi_c * valid).sum() / n_valid
    spec = np.argmax(aggregate, axis=1)
    present = np.zeros(4, bool)
    for a in range(4):
        present[a] = np.any(valid & (spec == a))
    unique = float(present.sum())
    phi_global = min(1.0, avg_phi * (0.5 + 0.5 * unique / 4.0))
    vert = (coh * valid).sum() / n_valid
    self_model = np.array([*global_arch, phi_global, vert], np.float32)
    return np.concatenate([cluster_full.reshape(-1), self_model]).astype(np.float32)



# revision 23
# speedup vs baseline: 1.1691x; 1.1691x over previous
"""Trainium2 Bass kernel for nn_BottomUpIntegrator (gnn_message_passing).

Sharding: cells split at cluster boundaries across 8 cores (2048 clusters
each). Per-core segmented reductions via one-hot scatter matmuls accumulating
into a single persistent PSUM bank with a core-invariant static window
schedule (W=8).

Phase A (per 8192-cell chunk, software-pipelined):
  DMA: feats fp8(e4m3) [37,2,4096] k-tiled for DoubleRow (bias folded as
       ones-row), host-packed bf16 [1|a|sur|a^2|eph] columns + one-hot window
       rows, one merged DMA.
  PE:  mm1 in fp8e4m3 DoubleRow (2 k-tiles of 37 rows, 0.5 cyc/row), mm2
       (base logits, hs-as-weights trick), scatter (vt12 host rows -> PSUM
       parts 32:42, ew rows -> parts 0:5).
  mm1 PSUM: ring of 2x[128,1536] tiles (6 banks); drains of 1536 cols
       (relu + fp32->bf16) alternate ACT/DVE into a 24-slot SBUF hsb ring,
       amortizing per-instruction access-latency overhead.
  ACT: tanh (sigmoid = .5+.5 tanh(v/2)), exp(w).
  Pool (SBUF-only): sigmoid affine, clip chain, ew*[1,a] scatter rows.
Phase B: strided transposes (batched 4-per-PSUM-tile, 4 wide copies),
  stats chain distributed across DVE/Pool/ACT, cluster MLP via one
  block-diagonal matmul (clusters stay on partitions; V1 bias via
  ones-feature, V2 dot via bf16 elementwise+reduce).
Organism-level finale (valid mask, argmax/present, softmax weights) runs on
host in f64 from per-cluster outputs + impc (host knows exact counts).
"""
import numpy as np
import ml_dtypes

import json as _json

from concourse import bass, mybir
from concourse import bass2jax as _b2j
from concourse import bass_utils as _bu
from concourse.tile import TileContext
from concourse.bass_utils import run_bass_kernel_spmd

_orig_compile = _bu.compile_bir_kernel


def _split_waits_compile(bir_json, tmpdir, neff_name="file.neff"):
    """Walrus lowers at most ONE semaphore wait per TPB instruction struct.
    Tile emits several. Hoist extras onto injected same-engine EventSemaphore
    wait instructions immediately before the owner (semantically identical:
    engines execute in program order)."""
    d = _json.loads(bir_json)
    cnt = 0
    for fn in d["functions"]:
        for blk in fn["blocks"]:
            newlist = []
            for ins in blk["instructions"]:
                si = ins.get("sync_info")
                waits = si.get("on_wait", []) if si else []
                if si and len(waits) > 1 and ins.get("opcode") not in (
                        "EventSemaphore",):
                    for w_i, w in enumerate(waits[:-1]):
                        cnt += 1
                        newlist.append({
                            "debug": ins.get("debug", 0),
                            "engine": ins["engine"],
                            "ins": [], "outs": [],
                            "name": f"{ins['name']}-wsplit{w_i}",
                            "opcode": "EventSemaphore",
                            "sync_info": {"on_update": [], "on_wait": [w]},
                        })
                    si["on_wait"] = [waits[-1]]
                newlist.append(ins)
            blk["instructions"] = newlist
    print(f"[wait-split] hoisted {cnt} extra waits")
    return _orig_compile(_json.dumps(d).encode(), tmpdir, neff_name=neff_name)


_bu.compile_bir_kernel = _split_waits_compile
_b2j.compile_bir_kernel = _split_waits_compile

F32 = mybir.dt.float32
BF16 = mybir.dt.bfloat16
FP8E4 = mybir.dt.float8e4
AF = mybir.ActivationFunctionType
OP = mybir.AluOpType
AX = mybir.AxisListType
PM = mybir.MatmulPerfMode

NCORES = 8
KLOC = 2048            # clusters per core
NPAD = 262144          # padded cells per core
CHUNK = 8192           # cells per chunk
NCHUNK = NPAD // CHUNK # 32
W = 8                  # onehot window width (clusters)
MARGIN = 2             # window start = clip(s - MARGIN, 0, 512 - W)
NTILES = NPAD // 128   # 2048 scatter tiles per core
TPB = NTILES // 4      # tiles per 512-cluster block
PADSEG = 1.0e9
NSLOT = NPAD // 2 // 512   # 256 global mm1 psum 512-col slots
HSB_SLOTS = 24             # hsb ring capacity in 512-col slots (3 chunks)

# scatter stationary rows, accumulated in one scat bank [48, 512]:
#  parts 0:5   <- vt5  (device): [ew, ew*a4]
#  parts 32:42 <- vt12 (host):   [1, a4, sur, a2_4]  (lhsT cols 0:10)
# vt12 (host) col layout: [1, a4, sur, a2_4, eph, pad]


def _window_start(S):
    s = S % TPB
    return int(np.clip(s - MARGIN, 0, 512 - W))


def build_program():
    nc = bass.Bass(trn_type="TRN2", use_seq_codegen=True)
    featsT = nc.dram_tensor("featsT", [37, 2, NPAD // 2], FP8E4,
                            kind="ExternalInput")
    vohd = nc.dram_tensor("vohd", [NCHUNK, 128, 768 + 64 * W], BF16,
                          kind="ExternalInput")
    w1d = nc.dram_tensor("w1d", [37, 2, 128], FP8E4, kind="ExternalInput")
    cbfd = nc.dram_tensor("cbfd", [128, 1154], BF16, kind="ExternalInput")
    cf32d = nc.dram_tensor("cf32d", [128, 131], F32, kind="ExternalInput")
    out_all = nc.dram_tensor("out_all", [128, 144], F32, kind="ExternalOutput")

    with TileContext(nc) as tc:
        with (
            tc.tile_pool(name="const", bufs=1) as cp,
            tc.tile_pool(name="feats", bufs=4) as fp,
            tc.tile_pool(name="small", bufs=4) as sp,
            tc.tile_pool(name="scatv", bufs=7) as vp,
            tc.tile_pool(name="ph_b", bufs=1) as bp,
        ):
            # ---- constants ----------------------------------------------
            w1s = cp.tile([37, 2, 128], FP8E4, tag="w1s")
            nc.sync.dma_start(out=w1s[:], in_=w1d[:])
            cbf = cp.tile([128, 1154], BF16, tag="cbf")
            nc.sync.dma_start(out=cbf[:], in_=cbfd[:])
            cf32 = cp.tile([128, 131], F32, tag="cf32")
            nc.sync.dma_start(out=cf32[:], in_=cf32d[:])
            w2bs = cbf[:, 0:2]
            v2rep = cbf[:, 2:514]
            v1blkF = cbf[:, 514:1026]
            idb_c = cbf[:, 1026:1154]
            b2hs = cf32[:, 0:1]
            c2h = cf32[:, 1:2]
            ids = cf32[:, 2:130]
            zbf = cp.tile([128, 512], BF16, tag="zbf")
            nc.vector.memset(zbf[:], 0.0)
            # hsb ring: 24 x 512-col slots of relu'd mm1 output (bf16)
            hsbring = cp.tile([128, HSB_SLOTS * 512], BF16, tag="hsbring")

            # Pre-touch DMA-loaded constants on their consuming engines so no
            # later compute instruction needs a second (DMA) semaphore wait.
            scra = cp.tile([128, 2], F32, tag="scra")
            nc.scalar.activation(out=scra[:, 0:1], in_=b2hs, func=AF.Copy)
            nc.scalar.activation(out=scra[:, 1:2], in_=c2h, func=AF.Copy)

            # persistent scatter accumulator: 1 PSUM bank [48, 512]
            pps_cm = tc.tile_pool(name="scatps", bufs=1, space="PSUM")
            pps = pps_cm.__enter__()
            scat = pps.tile([48, 512], F32, tag="scat", name="scat")
            # PE touch of PE-consumed consts (rides on Ldweights; overwritten
            # by the zeroing matmul below).
            nc.tensor.matmul(out=scat[0:1, 0:1], lhsT=ids[0:1, 0:1],
                             rhs=ids[0:1, 0:1], start=True, stop=True,
                             skip_group_check=True)
            nc.tensor.matmul(out=scat[0:1, 0:2], lhsT=w1s[0:1, 0, 0:1],
                             rhs=w1s[0:1, 0, 0:2], start=True, stop=True,
                             skip_group_check=True)
            nc.tensor.matmul(out=scat[0:2, 0:2], lhsT=w2bs[0:1, :],
                             rhs=w2bs[0:1, :], start=True, stop=True,
                             skip_group_check=True)
            nc.tensor.matmul(out=scat[:], lhsT=zbf[:, 0:48], rhs=zbf[:],
                             start=True, stop=False, skip_group_check=True)
            sc = bp.tile([48, 2048], F32, tag="sc")

            # ---- phase A ------------------------------------------------
            with (
                tc.tile_pool(name="ringps", bufs=1, space="PSUM") as ppr,
                tc.tile_pool(name="mm2ps", bufs=1, space="PSUM") as pp2,
            ):
                # ring: 6 psum banks of mm1 output split over 3 tiles; a tile
                # drains as soon as filled while the other two fill/drain, so
                # no engine waits on its own tile's refill.
                RSL = [2, 2, 2]
                LAP = sum(RSL)
                rg = [ppr.tile([128, 512 * n], F32, tag=f"rg{i}", name=f"rg{i}")
                      for i, n in enumerate(RSL)]
                RTILE = []      # lap position -> (tile idx, local slot)
                for i, n in enumerate(RSL):
                    for j in range(n):
                        RTILE.append((i, j))
                dctr = [0]      # global drain counter (engine alternation)

                # software pipeline: at iteration it --
                #   DMA(it) | mm1+drain(it-1) | mm2+chain(it-2) | scatter(it-3)
                T = {}

                def dma_stage(k):
                    ft = fp.tile([37, 2, 4096], FP8E4, tag="ft")
                    nc.sync.dma_start(out=ft[:],
                                      in_=featsT[:, :, k * 4096:(k + 1) * 4096])
                    voh = vp.tile([128, 768 + 64 * W], BF16, tag="voh")
                    nc.sync.dma_start(out=voh[:], in_=vohd[k])
                    T[k] = dict(ft=ft, vt12=voh[:, 0:768], oh=voh[:, 768:])

                def _drain(ti, g_first, nslots):
                    # drain first nslots of ring tile ti; g_first = global mm1
                    # slot index of the tile's first slot (fixes hsb position)
                    cols = 512 * nslots
                    src = rg[ti][:, 0:cols]
                    c0 = (g_first % HSB_SLOTS) * 512
                    dst = hsbring[:, c0:c0 + cols]
                    if dctr[0] % 2 == 0:
                        nc.scalar.activation(out=dst, in_=src, func=AF.Relu)
                    else:
                        nc.vector.tensor_scalar(out=dst, in0=src, scalar1=0.0,
                                                scalar2=None, op0=OP.max)
                    dctr[0] += 1

                def mm1_stage(k, half):
                    d = T[k]
                    for q in range(4 * half, 4 * half + 4):
                        g = 8 * k + q
                        ti, loc = RTILE[g % LAP]
                        nc.tensor.matmul(
                            out=rg[ti][:, 512 * loc:512 * (loc + 1)],
                            lhsT=w1s[:],
                            rhs=d["ft"][:, :, 512 * q:512 * (q + 1)],
                            start=True, stop=True, perf_mode=PM.DoubleRow,
                            skip_group_check=True)
                        if loc == RSL[ti] - 1:
                            _drain(ti, g - loc, RSL[ti])
                        elif g == NSLOT - 1:
                            _drain(ti, g - loc, loc + 1)  # tail flush

                P = {}   # per 2-chunk-pair state (tanh/exp batched: ACT init
                         # overhead halves)

                def mm2_stage(k):
                    p, c = k // 2, k % 2
                    if c == 0:
                        P[p] = dict(bb=pp2.tile([128, 128], F32, tag="bb", name="bb"))
                    bb = P[p]["bb"]
                    base = ((8 * k) % HSB_SLOTS) * 512
                    for tt in range(32):
                        nc.tensor.matmul(
                            out=bb[:, 64 * c + 2 * tt:64 * c + 2 * tt + 2],
                            lhsT=hsbring[:, base + 128 * tt:base + 128 * (tt + 1)],
                            rhs=w2bs[:],
                            start=(tt == 0 and c == 0),
                            stop=(tt == 31 and c == 1),
                            skip_group_check=True)

                def chain_t(p):
                    # tanh for both chunks of pair p
                    bb = P[p]["bb"]
                    th = sp.tile([128, 128], F32, tag="th", name="th")
                    nc.scalar.activation(out=th[:], in_=bb[:], func=AF.Tanh,
                                         bias=b2hs, scale=0.5)
                    # xw cols: [xv_e | xv_o | wv_e | wv_o]
                    P[p].update(th=th, xw=sp.tile([128, 256], F32, tag="xw", name="xw"))

                def chain_a(k):
                    # sig(v) = .5 + .5*tanh(.5*v); x = max(sig*eph, .01)
                    # (sig*eph < 1 always); w = x*eph; vt12 col 10 = eph
                    p, c = k // 2, k % 2
                    vt12v = T[k]["vt12"].rearrange("p (s v) -> p s v", v=12)
                    th = P[p]["th"][:, 64 * c:64 * c + 64]
                    xw = P[p]["xw"]
                    xv = xw[:, 64 * c:64 * c + 64]
                    wv = xw[:, 128 + 64 * c:192 + 64 * c]
                    eph = vt12v[:, :, 10:11]
                    sg = sp.tile([128, 64], F32, tag="sg", name="sg")
                    nc.gpsimd.tensor_scalar(out=sg[:], in0=th, scalar1=0.5,
                                            scalar2=0.5, op0=OP.mult, op1=OP.add)
                    nc.gpsimd.tensor_tensor(
                        out=xv.rearrange("p (s o) -> p s o", o=1),
                        in0=sg[:].rearrange("p (s o) -> p s o", o=1),
                        in1=eph, op=OP.mult)
                    nc.gpsimd.tensor_scalar(out=xv, in0=xv, scalar1=0.01,
                                            scalar2=None, op0=OP.max)
                    nc.gpsimd.tensor_tensor(
                        out=wv.rearrange("p (s o) -> p s o", o=1),
                        in0=xv.rearrange("p (s o) -> p s o", o=1),
                        in1=eph, op=OP.mult)

                def chain_e(p):
                    # exp for both chunks of pair p
                    ew = sp.tile([128, 128], BF16, tag="ewt", name="ewt")
                    nc.scalar.activation(out=ew[:], in_=P[p]["xw"][:, 128:256],
                                         func=AF.Exp)
                    P[p]["ew"] = ew

                def chain_b(k):
                    p, c = k // 2, k % 2
                    d = T[k]
                    vt12v = d["vt12"].rearrange("p (s v) -> p s v", v=12)
                    ewt = P[p]["ew"][:, 64 * c:64 * c + 64]
                    vt5 = vp.tile([128, 5 * 64], BF16, tag="vt5")
                    v5 = vt5[:].rearrange("p (s v) -> p s v", v=5)
                    nc.gpsimd.tensor_tensor(out=v5,
                                            in0=vt12v[:, :, 0:5],
                                            in1=ewt.to_broadcast([128, 64, 5]),
                                            op=OP.mult)
                    d["vt5"] = vt5
                    if c == 1:
                        del P[p]

                def scatter_stage(k):
                    # col j -> sorted tile S = 64k + 32*(j%2) + j//2
                    d = T[k]
                    vt12 = d["vt12"]
                    vt5 = d["vt5"]
                    for j in range(64):
                        S = 64 * k + 32 * (j % 2) + (j // 2)
                        f = _window_start(S)
                        last = (k % 8 == 7 and j >= 62)
                        nc.tensor.matmul(out=scat[0:5, f:f + W],
                                         lhsT=vt5[:, 5 * j:5 * j + 5],
                                         rhs=d["oh"][:, W * j:W * j + W],
                                         start=False, stop=last,
                                         skip_group_check=True)
                        nc.tensor.matmul(out=scat[32:42, f:f + W],
                                         lhsT=vt12[:, 12 * j:12 * j + 10],
                                         rhs=d["oh"][:, W * j:W * j + W],
                                         start=False, stop=last,
                                         skip_group_check=True)
                    if k % 8 == 7:
                        blk = k // 8
                        dst = sc[:, 512 * blk:512 * (blk + 1)]
                        if blk % 2 == 0:
                            nc.vector.tensor_copy(out=dst, in_=scat[:])
                        else:
                            nc.scalar.activation(out=dst, in_=scat[:],
                                                 func=AF.Copy)
                    # free stale per-chunk state
                    del T[k]

                def zero_stage():
                    # re-zero scat bank; deferred one iteration past the sc
                    # copy so its WAR wait cannot head-of-line block PE's mm1s
                    nc.tensor.matmul(out=scat[:], lhsT=zbf[:, 0:48],
                                     rhs=zbf[:], start=True, stop=False,
                                     skip_group_check=True)

                # tile_wait_until slots pin the Tile scheduler to this exact
                # per-engine interleave (ACT: drain, tanh, drain, exp) --
                # left alone it queues exp right after tanh, and the Pool
                # chain latency then stalls the whole ACT stream.
                for it in range(NCHUNK + 7):
                    if it < NCHUNK:
                        with tc.tile_wait_until(it + 0.0):
                            dma_stage(it)
                    if 2 <= it <= NCHUNK + 1:
                        with tc.tile_wait_until(it + 0.1):
                            mm1_stage(it - 2, 0)
                    if 3 <= it <= NCHUNK + 2:
                        with tc.tile_wait_until(it + 0.2):
                            mm2_stage(it - 3)
                    if 4 <= it <= NCHUNK + 3 and (it - 4) % 2 == 0:
                        with tc.tile_wait_until(it + 0.3):
                            chain_t((it - 4) // 2)
                    if 2 <= it <= NCHUNK + 1:
                        with tc.tile_wait_until(it + 0.4):
                            mm1_stage(it - 2, 1)
                    if 4 <= it <= NCHUNK + 3:
                        with tc.tile_wait_until(it + 0.45):
                            chain_a(it - 4)
                    if 5 <= it <= NCHUNK + 4 and (it - 5) % 2 == 0:
                        with tc.tile_wait_until(it + 0.5):
                            chain_e((it - 5) // 2)
                    if 5 <= it <= NCHUNK + 4:
                        with tc.tile_wait_until(it + 0.55):
                            chain_b(it - 5)
                    if 7 <= it <= NCHUNK + 6 and (it - 7) % 8 == 7 and \
                            (it - 7) // 8 < 3:
                        with tc.tile_wait_until(it + 0.58):
                            zero_stage()
                    if 6 <= it <= NCHUNK + 5:
                        with tc.tile_wait_until(it + 0.6):
                            scatter_stage(it - 6)

            # ---- phase B ------------------------------------------------
            # sc rows: 0=sew, 1:5=sewa, 32=cnt, 33:37=sa, 37=ssur, 38:42=ssq
            tc.strict_bb_all_engine_barrier()
            pps_cm.__exit__(None, None, None)

            with (
                tc.tile_pool(name="ptps", bufs=2, space="PSUM") as ppt,
                tc.tile_pool(name="ptcs", bufs=1, space="PSUM") as pptc,
                tc.tile_pool(name="mmbps", bufs=1, space="PSUM") as ppm,
            ):
                # batched transposes: 4 x [128,48] per PSUM tile, 1 wide copy
                tt = bp.tile([128, 16 * 48], F32, tag="tt")
                scv = sc[:].rearrange("p (c g) -> p g c", g=16)
                for b4 in range(4):
                    pt = ppt.tile([128, 192], F32, tag="pt")
                    for j in range(4):
                        nc.tensor.transpose(out=pt[:, 48 * j:48 * (j + 1)],
                                            in_=scv[:, 4 * b4 + j, :],
                                            identity=ids[0:48, 0:48])
                    if b4 % 2 == 0:
                        nc.vector.tensor_copy(out=tt[:, 192 * b4:192 * (b4 + 1)],
                                              in_=pt[:])
                    else:
                        nc.scalar.activation(out=tt[:, 192 * b4:192 * (b4 + 1)],
                                             in_=pt[:], func=AF.Copy)
                tv = tt[:].rearrange("p (b q) -> p b q", q=48)
                cnt = tv[:, :, 32:33]    # [128,16,1]
                sa = tv[:, :, 33:37]
                ssur = tv[:, :, 37:38]
                ssq = tv[:, :, 38:42]
                sew = tv[:, :, 0:1]
                sewa = tv[:, :, 1:5]

                def wt(tag):
                    return bp.tile([128, 16], F32, tag=tag, name=tag)

                def v3(t):
                    return t[:].rearrange("p (b a) -> p b a", a=1)

                def w4(tag):
                    t = bp.tile([128, 64], F32, tag=tag, name=tag)
                    return t, t[:].rearrange("p (b a) -> p b a", a=4)

                # batched reciprocals: [cntc | den | cm1] -> one reciprocal
                r3 = bp.tile([128, 48], F32, tag="r3")
                nc.vector.tensor_scalar(
                    out=r3[:, 0:16].rearrange("p (b a) -> p b a", a=1),
                    in0=cnt, scalar1=1.0, scalar2=None, op0=OP.max)
                nc.gpsimd.tensor_scalar(
                    out=r3[:, 16:32].rearrange("p (b a) -> p b a", a=1),
                    in0=sew, scalar1=1.0, scalar2=None, op0=OP.max)
                nc.gpsimd.tensor_scalar(
                    out=r3[:, 32:48].rearrange("p (b a) -> p b a", a=1),
                    in0=cnt, scalar1=-1.0, scalar2=1.0, op0=OP.add, op1=OP.max)
                rr = bp.tile([128, 48], F32, tag="rr")
                nc.vector.reciprocal(out=rr[:], in_=r3[:])
                rc = rr[:, 0:16]
                rden = rr[:, 16:32]
                rcm1 = rr[:, 32:48]

                # softmax/aggregate branch (DVE + ACT)
                agr, agrv = w4("agr")
                nc.vector.tensor_tensor(out=agrv, in0=sewa,
                                        in1=rden.to_broadcast([128, 16, 4]),
                                        op=OP.mult)
                es, esv = w4("es")
                nc.scalar.activation(out=es[:], in_=agr[:], func=AF.Exp)
                ssum = wt("ssum")
                nc.vector.tensor_reduce(out=v3(ssum), in_=esv, axis=AX.X,
                                        op=OP.add)
                rssum = wt("rssum")
                nc.vector.reciprocal(out=rssum[:], in_=ssum[:])
                agg, aggv = w4("agg")
                nc.vector.tensor_tensor(out=aggv, in0=esv,
                                        in1=rssum[:].to_broadcast([128, 16, 4]),
                                        op=OP.mult)

                # variance branch (Pool + DVE); mean^2*cnt = mean*sa, and the
                # rcm1 scale commutes past the reduce
                mean, meanv = w4("mean")
                nc.gpsimd.tensor_tensor(out=meanv, in0=sa,
                                        in1=rc.to_broadcast([128, 16, 4]),
                                        op=OP.mult)
                m2, m2v = w4("m2")
                nc.gpsimd.tensor_tensor(out=m2v, in0=meanv, in1=sa,
                                        op=OP.mult)
                varn, varnv = w4("varn")
                nc.vector.tensor_tensor(out=varnv, in0=ssq, in1=m2v,
                                        op=OP.subtract)
                vmr = wt("vmr")
                nc.vector.tensor_reduce(out=v3(vmr), in_=varnv, axis=AX.X,
                                        op=OP.add)
                vm4 = wt("vm4")
                nc.vector.tensor_tensor(out=vm4[:], in0=vmr[:],
                                        in1=rcm1, op=OP.mult)
                # phic = 1 - min(1, 2*vm) = relu(1 - 0.5*vm4)   (vm = vm4/4)
                phic = wt("phic")
                nc.scalar.activation(out=phic[:], in_=vm4[:], func=AF.Relu,
                                     bias=1.0, scale=-0.5)
                coh = wt("coh")
                nc.vector.tensor_scalar(out=coh[:], in0=vm4[:], scalar1=-0.25,
                                        scalar2=1.0, op0=OP.mult, op1=OP.add)
                perr = wt("perr")
                nc.gpsimd.tensor_tensor(out=v3(perr), in0=ssur,
                                        in1=rc.rearrange("p (b a) -> p b a", a=1),
                                        op=OP.mult)
                integ = wt("integ")
                nc.vector.tensor_scalar(out=integ[:], in0=perr[:], scalar1=-1.0,
                                        scalar2=1.0, op0=OP.mult, op1=OP.add)
                nc.vector.tensor_tensor(out=integ[:], in0=integ[:], in1=phic[:],
                                        op=OP.mult)
                szf = wt("szf")
                nc.gpsimd.tensor_scalar(out=v3(szf), in0=cnt, scalar1=0.05,
                                        scalar2=1.0, op0=OP.mult, op1=OP.min)

                # cluster MLP: cft [128, 16 groups x 8 feats] bf16,
                # hc = relu(cft @ v1blkF) via one block-diagonal matmul
                cft = bp.tile([128, 128], BF16, tag="cft")
                cfv = cft[:].rearrange("p (b q) -> p b q", q=8)
                nc.vector.tensor_copy(out=cfv[:, :, 0:4], in_=aggv)
                nc.scalar.activation(
                    out=cfv[:, :, 4:5],
                    in_=phic[:].rearrange("p (b a) -> p b a", a=1),
                    func=AF.Copy)
                nc.vector.tensor_copy(out=cfv[:, :, 5:6],
                                      in_=coh[:].to_broadcast([128, 16, 1]))
                nc.scalar.activation(
                    out=cfv[:, :, 6:7],
                    in_=szf[:].rearrange("p (b a) -> p b a", a=1),
                    func=AF.Copy)
                nc.gpsimd.memset(cfv[:, :, 7:8], 1.0)
                cfT = pptc.tile([128, 128], BF16, tag="cfT")
                nc.tensor.transpose(out=cfT[:], in_=cft[:], identity=idb_c)
                cftSB = bp.tile([128, 128], BF16, tag="cftSB")
                nc.vector.tensor_copy(out=cftSB[:], in_=cfT[:])
                hcp = ppm.tile([128, 512], F32, tag="hcp")
                nc.tensor.matmul(out=hcp[:], lhsT=cftSB[:], rhs=v1blkF,
                                 start=True, stop=True, skip_group_check=True)
                hcsb = bp.tile([128, 512], BF16, tag="hcsb")
                nc.scalar.activation(out=hcsb[:], in_=hcp[:], func=AF.Relu)
                hv2 = bp.tile([128, 512], BF16, tag="hv2")
                nc.vector.tensor_tensor(out=hv2[:], in0=hcsb[:], in1=v2rep,
                                        op=OP.mult)
                bb2 = wt("bb2")
                nc.vector.tensor_reduce(
                    out=v3(bb2),
                    in_=hv2[:].rearrange("p (b h) -> p b h", h=32),
                    axis=AX.X, op=OP.add)
                basec = wt("basec")
                nc.scalar.activation(out=basec[:], in_=bb2[:], func=AF.Tanh,
                                     bias=c2h, scale=0.5)
                nc.vector.tensor_scalar(out=basec[:], in0=basec[:], scalar1=0.5,
                                        scalar2=0.5, op0=OP.mult, op1=OP.add)

                # cluster_out [2048, 8] + impc, one [128, 144] output tile
                oc = bp.tile([128, 144], F32, tag="oc")
                ocv = oc[:, 0:128].rearrange("p (b q) -> p b q", q=8)
                impc = oc[:, 128:144]
                nc.vector.tensor_tensor(out=impc, in0=basec[:], in1=phic[:],
                                        op=OP.mult)
                nc.vector.tensor_scalar(out=impc, in0=impc, scalar1=0.01,
                                        scalar2=1.0, op0=OP.max, op1=OP.min)
                nc.vector.tensor_copy(out=ocv[:, :, 0:4], in_=aggv)
                nc.scalar.activation(
                    out=ocv[:, :, 4:5],
                    in_=phic[:].rearrange("p (b a) -> p b a", a=1),
                    func=AF.Copy)
                nc.vector.tensor_copy(out=ocv[:, :, 5:6],
                                      in_=coh[:].to_broadcast([128, 16, 1]))
                nc.scalar.activation(
                    out=ocv[:, :, 6:7],
                    in_=perr[:].rearrange("p (b a) -> p b a", a=1),
                    func=AF.Copy)
                nc.vector.tensor_copy(out=ocv[:, :, 7:8],
                                      in_=integ[:].to_broadcast([128, 16, 1]))
                nc.sync.dma_start(out=out_all[:], in_=oc[:])
    return nc


_NC_CACHE = None


def _get_program():
    global _NC_CACHE
    if _NC_CACHE is None:
        _NC_CACHE = build_program()
    return _NC_CACHE


def _host_prep_core(c, state, arch, energy, phi_local, surprise, seg_ids):
    B0 = int(np.searchsorted(seg_ids, 2048 * c))
    B1 = int(np.searchsorted(seg_ids, 2048 * (c + 1)))
    Nc = B1 - B0
    lseg = (seg_ids[B0:B1] - 2048 * c).astype(np.int64)
    idx = np.full(NPAD, -1, np.int64)
    rel = np.full(NPAD, PADSEG, np.float32)
    cur = 0
    for S in range(NTILES):
        blk = S // TPB
        f = _window_start(S)
        wlo = 512 * blk + f
        whi = wlo + W
        take = min(128, int(np.searchsorted(lseg, whi)) - cur)
        if take > 0:
            assert lseg[cur] >= wlo, f"core {c} tile {S}: behind-lag"
            sl = np.arange(cur, cur + take)
            idx[S * 128:S * 128 + take] = sl
            rel[S * 128:S * 128 + take] = (lseg[sl] - wlo).astype(np.float32)
            cur += take
    assert cur == Nc, f"core {c}: {Nc - cur} cells not scheduled"
    m = idx >= 0

    def g(x):
        out = np.zeros((NPAD,) + x.shape[1:], np.float32)
        out[m] = x[B0:B1][idx[m]]
        return out

    return g(state), g(arch), g(energy), g(phi_local), g(surprise), rel, m


def _swz(x):
    """[NPAD, Q] cell-major -> [NCHUNK, 128, 64*Q] device layout."""
    Q = x.shape[1]
    return np.ascontiguousarray(
        x.reshape(NCHUNK, 2, 32, 128, Q).transpose(0, 3, 2, 1, 4).reshape(
            NCHUNK, 128, 64 * Q))


def kernel(state, arch, energy, phi_local, surprise, seg_ids, n_clusters,
           W1, b1, W2, b2, V1, c1, V2, c2):
    state = np.asarray(state, np.float32)
    arch = np.asarray(arch, np.float32)
    energy = np.asarray(energy, np.float32)
    phi_local = np.asarray(phi_local, np.float32)
    surprise = np.asarray(surprise, np.float32)
    seg_ids = np.asarray(seg_ids)
    W1 = np.asarray(W1, np.float32); b1 = np.asarray(b1, np.float32)
    W2 = np.asarray(W2, np.float32); b2 = np.asarray(b2, np.float32)
    V1 = np.asarray(V1, np.float32); c1 = np.asarray(c1, np.float32)
    V2 = np.asarray(V2, np.float32); c2 = np.asarray(c2, np.float32)

    w1f = np.zeros((74, 128), np.float32)
    w1f[0:36, 0:64] = W1
    w1f[36:72, 64:128] = W1
    w1f[72, 0:64] = b1
    w1f[72, 64:128] = b1
    w1k = np.ascontiguousarray(
        w1f.reshape(2, 37, 128).transpose(1, 0, 2))
    w2f = np.zeros((128, 2), np.float32)
    w2f[0:64, 0] = W2[:, 0]
    w2f[64:128, 1] = W2[:, 0]
    v1p = np.concatenate([V1, c1.reshape(1, 32)], 0)   # [8, 32]
    v1blkF = np.zeros((128, 512), np.float32)
    for gidx in range(16):
        v1blkF[8 * gidx:8 * gidx + 8, 32 * gidx:32 * gidx + 32] = v1p
    cbf = np.zeros((128, 1154), np.float32)
    cbf[:, 0:2] = w2f
    cbf[:, 2:514] = np.tile(V2[:, 0], (128, 16))
    cbf[:, 514:1026] = v1blkF
    cbf[:, 1026:1154] = np.eye(128, dtype=np.float32)
    cf32 = np.zeros((128, 131), np.float32)
    cf32[:, 0] = 0.5 * b2[0]
    cf32[:, 1] = 0.5 * c2[0]
    cf32[:, 2:130] = np.eye(128, dtype=np.float32)
    cf32[:, 130] = 1.0
    consts = dict(
        w1d=w1k.astype(ml_dtypes.float8_e4m3),
        cbfd=cbf.astype(ml_dtypes.bfloat16),
        cf32d=cf32,
    )
    iw = np.arange(W, dtype=np.float32)

    def _prep(c):
        st, ar, en, ph, su, rel, msk = _host_prep_core(
            c, state, arch, energy, phi_local, surprise, seg_ids)
        f36 = np.concatenate([st.T, ar.T], 0)              # [36, NPAD]
        f74 = np.concatenate(
            [f36.reshape(36, NCHUNK, 2, 4096).transpose(2, 0, 1, 3).reshape(
                72, NPAD // 2),
             np.ones((1, NPAD // 2), np.float32),
             np.zeros((1, NPAD // 2), np.float32)], 0)
        featsT = np.ascontiguousarray(
            f74.reshape(2, 37, NPAD // 2).transpose(1, 0, 2)).astype(
                ml_dtypes.float8_e4m3)
        # vt12: [1(mask), a4, sur, a2_4, eph, pad]
        vt12 = np.zeros((NPAD, 12), np.float32)
        vt12[:, 0] = msk
        vt12[:, 1:5] = ar
        vt12[:, 5] = su
        vt12[:, 6:10] = ar * ar
        vt12[:, 10] = en * ph
        oh = (rel[:, None] == iw[None, :]).astype(np.float32)   # [NPAD, W]
        voh = np.concatenate([_swz(vt12), _swz(oh)], axis=2)
        return dict(featsT=featsT,
                    vohd=np.ascontiguousarray(voh).astype(ml_dtypes.bfloat16),
                    **consts)

    from concurrent.futures import ThreadPoolExecutor
    with ThreadPoolExecutor(NCORES) as ex:
        in_maps = list(ex.map(_prep, range(NCORES)))
    nc = _get_program()
    res = run_bass_kernel_spmd(nc, in_maps, list(range(NCORES)))
    global LAST_RESULT
    LAST_RESULT = res
    outs = res.results
    alls = [np.asarray(outs[c]["out_all"]) for c in range(NCORES)]
    couts = [a[:, 0:128].reshape(2048, 8) for a in alls]
    impcs = [a[:, 128:144].reshape(-1) for a in alls]
    cluster_full = np.concatenate(couts, 0).astype(np.float32)
    impc = np.concatenate(impcs, 0).astype(np.float64)

    # organism-level finale on host (exact, f64)
    K = 16384
    counts = np.bincount(seg_ids, minlength=K)
    valid = counts > 0
    n_valid = max(float(valid.sum()), 1.0)
    aggregate = cluster_full[:, 0:4].astype(np.float64)
    phi_c = cluster_full[:, 4].astype(np.float64)
    coh = cluster_full[:, 5].astype(np.float64)
    iv = np.where(valid, impc, -np.inf)
    e = np.exp(iv - iv.max())
    wc = e / e.sum()
    ga = (wc[:, None] * aggregate).sum(0)
    eg = np.exp(ga - ga.max())
    global_arch = (eg / eg.sum()).astype(np.float32)
    avg_phi = (phi_c * valid).sum() / n_valid
    spec = np.argmax(aggregate, axis=1)
    present = np.zeros(4, bool)
    for a in range(4):
        present[a] = np.any(valid & (spec == a))
    unique = float(present.sum())
    phi_global = min(1.0, avg_phi * (0.5 + 0.5 * unique / 4.0))
    vert = (coh * valid).sum() / n_valid
    self_model = np.array([*global_arch, phi_global, vert], np.float32)
    return np.concatenate([cluster_full.reshape(-1), self_model]).astype(np.float32)


# revision 35
# speedup vs baseline: 1.1984x; 1.0251x over previous
"""Trainium2 Bass kernel for nn_BottomUpIntegrator (gnn_message_passing).

Sharding: cells split at cluster boundaries across 8 cores (2048 clusters
each). Per-core segmented reductions via one-hot scatter matmuls accumulating
into a single persistent PSUM bank with a core-invariant static window
schedule (W=8).

Phase A (per 8192-cell chunk, software-pipelined):
  DMA: feats fp8(e4m3) [37,2,4096] k-tiled for DoubleRow (bias folded as
       ones-row), host-packed bf16 [1|a|sur|a^2|eph] columns + one-hot window
       rows, one merged DMA.
  PE:  mm1 in fp8e4m3 DoubleRow (2 k-tiles of 37 rows, 0.5 cyc/row), mm2
       (base logits, hs-as-weights trick), scatter (vt12 host rows -> PSUM
       parts 32:42, ew rows -> parts 0:5).
  mm1 PSUM: ring of 2x[128,1536] tiles (6 banks); drains of 1536 cols
       (relu + fp32->bf16) alternate ACT/DVE into a 24-slot SBUF hsb ring,
       amortizing per-instruction access-latency overhead.
  ACT: tanh (sigmoid = .5+.5 tanh(v/2)), exp(w).
  Pool (SBUF-only): sigmoid affine, clip chain, ew*[1,a] scatter rows.
Phase B: strided transposes (batched 4-per-PSUM-tile, 4 wide copies),
  stats chain distributed across DVE/Pool/ACT, cluster MLP via one
  block-diagonal matmul (clusters stay on partitions; V1 bias via
  ones-feature, V2 dot via bf16 elementwise+reduce).
Organism-level finale (valid mask, argmax/present, softmax weights) runs on
host in f64 from per-cluster outputs + impc (host knows exact counts).
"""
import numpy as np
import ml_dtypes

import json as _json

from concourse import bass, mybir
from concourse import bass2jax as _b2j
from concourse import bass_utils as _bu
from concourse.tile import TileContext
from concourse.bass_utils import run_bass_kernel_spmd

_orig_compile = _bu.compile_bir_kernel


def _split_waits_compile(bir_json, tmpdir, neff_name="file.neff"):
    """Walrus lowers at most ONE semaphore wait per TPB instruction struct.
    Tile emits several. Hoist extras onto injected same-engine EventSemaphore
    wait instructions immediately before the owner (semantically identical:
    engines execute in program order)."""
    d = _json.loads(bir_json)
    cnt = 0
    for fn in d["functions"]:
        for blk in fn["blocks"]:
            newlist = []
            for ins in blk["instructions"]:
                si = ins.get("sync_info")
                waits = si.get("on_wait", []) if si else []
                if si and len(waits) > 1 and ins.get("opcode") not in (
                        "EventSemaphore",):
                    for w_i, w in enumerate(waits[:-1]):
                        cnt += 1
                        newlist.append({
                            "debug": ins.get("debug", 0),
                            "engine": ins["engine"],
                            "ins": [], "outs": [],
                            "name": f"{ins['name']}-wsplit{w_i}",
                            "opcode": "EventSemaphore",
                            "sync_info": {"on_update": [], "on_wait": [w]},
                        })
                    si["on_wait"] = [waits[-1]]
                newlist.append(ins)
            blk["instructions"] = newlist
    print(f"[wait-split] hoisted {cnt} extra waits")
    return _orig_compile(_json.dumps(d).encode(), tmpdir, neff_name=neff_name)


_bu.compile_bir_kernel = _split_waits_compile
_b2j.compile_bir_kernel = _split_waits_compile

F32 = mybir.dt.float32
BF16 = mybir.dt.bfloat16
FP8E4 = mybir.dt.float8e4
AF = mybir.ActivationFunctionType
OP = mybir.AluOpType
AX = mybir.AxisListType
PM = mybir.MatmulPerfMode

NCORES = 8
KLOC = 2048            # clusters per core
NPAD = 262144          # padded cells per core
CHUNK = 8192           # cells per chunk
NCHUNK = NPAD // CHUNK # 32
W = 8                  # onehot window width (clusters)
MARGIN = 2             # window start = clip(s - MARGIN, 0, 512 - W)
NTILES = NPAD // 128   # 2048 scatter tiles per core
TPB = NTILES // 4      # tiles per 512-cluster block
PADSEG = 1.0e9
NSLOT = NPAD // 2 // 512   # 256 global mm1 psum 512-col slots
HSB_SLOTS = 24             # hsb ring capacity in 512-col slots (3 chunks)

# scatter stationary rows, accumulated in one scat bank [48, 512]:
#  parts 0:5   <- vt5  (device): [ew, ew*a4]
#  parts 32:42 <- vt12 (host):   [1, a4, sur, a2_4]  (lhsT cols 0:10)
# vt12 (host) col layout: [1, a4, sur, a2_4, eph, pad]


def _window_start(S):
    s = S % TPB
    return int(np.clip(s - MARGIN, 0, 512 - W))


def build_program():
    nc = bass.Bass(trn_type="TRN2", use_seq_codegen=True)
    featsT = nc.dram_tensor("featsT", [37, 2, NPAD // 2], FP8E4,
                            kind="ExternalInput")
    vohd = nc.dram_tensor("vohd", [NCHUNK, 128, 768 + 64 * W], BF16,
                          kind="ExternalInput")
    w1d = nc.dram_tensor("w1d", [37, 2, 128], FP8E4, kind="ExternalInput")
    cbfd = nc.dram_tensor("cbfd", [128, 1154], BF16, kind="ExternalInput")
    cf32d = nc.dram_tensor("cf32d", [128, 131], F32, kind="ExternalInput")
    out_all = nc.dram_tensor("out_all", [128, 144], F32, kind="ExternalOutput")

    with TileContext(nc) as tc:
        with (
            tc.tile_pool(name="const", bufs=1) as cp,
            tc.tile_pool(name="feats", bufs=4) as fp,
            tc.tile_pool(name="small", bufs=4) as sp,
            tc.tile_pool(name="scatv", bufs=7) as vp,
            tc.tile_pool(name="ph_b", bufs=1) as bp,
        ):
            # ---- constants ----------------------------------------------
            w1s = cp.tile([37, 2, 128], FP8E4, tag="w1s")
            nc.sync.dma_start(out=w1s[:], in_=w1d[:])
            cbf = cp.tile([128, 1154], BF16, tag="cbf")
            nc.sync.dma_start(out=cbf[:], in_=cbfd[:])
            cf32 = cp.tile([128, 131], F32, tag="cf32")
            nc.sync.dma_start(out=cf32[:], in_=cf32d[:])
            w2bs = cbf[:, 0:2]
            v2rep = cbf[:, 2:514]
            v1blkF = cbf[:, 514:1026]
            idb_c = cbf[:, 1026:1154]
            b2hs = cf32[:, 0:1]
            c2h = cf32[:, 1:2]
            ids = cf32[:, 2:130]
            zbf = cp.tile([128, 512], BF16, tag="zbf")
            nc.vector.memset(zbf[:], 0.0)
            # hsb ring: 24 x 512-col slots of relu'd mm1 output (bf16)
            hsbring = cp.tile([128, HSB_SLOTS * 512], BF16, tag="hsbring")

            # Pre-touch DMA-loaded constants on their consuming engines so no
            # later compute instruction needs a second (DMA) semaphore wait.
            scra = cp.tile([128, 2], F32, tag="scra")
            nc.scalar.activation(out=scra[:, 0:1], in_=b2hs, func=AF.Copy)
            nc.scalar.activation(out=scra[:, 1:2], in_=c2h, func=AF.Copy)

            # persistent scatter accumulator: 1 PSUM bank [48, 512]
            pps_cm = tc.tile_pool(name="scatps", bufs=1, space="PSUM")
            pps = pps_cm.__enter__()
            scat = pps.tile([48, 512], F32, tag="scat", name="scat")
            # PE touch of PE-consumed consts (rides on Ldweights; overwritten
            # by the zeroing matmul below).
            nc.tensor.matmul(out=scat[0:1, 0:1], lhsT=ids[0:1, 0:1],
                             rhs=ids[0:1, 0:1], start=True, stop=True,
                             skip_group_check=True)
            nc.tensor.matmul(out=scat[0:1, 0:2], lhsT=w1s[0:1, 0, 0:1],
                             rhs=w1s[0:1, 0, 0:2], start=True, stop=True,
                             skip_group_check=True)
            nc.tensor.matmul(out=scat[0:2, 0:2], lhsT=w2bs[0:1, :],
                             rhs=w2bs[0:1, :], start=True, stop=True,
                             skip_group_check=True)
            nc.tensor.matmul(out=scat[:], lhsT=zbf[:, 0:48], rhs=zbf[:],
                             start=True, stop=False, skip_group_check=True)
            sc = bp.tile([48, 2048], F32, tag="sc")

            # ---- phase A ------------------------------------------------
            with (
                tc.tile_pool(name="ringps", bufs=1, space="PSUM") as ppr,
                tc.tile_pool(name="mm2ps", bufs=1, space="PSUM") as pp2,
            ):
                # ring: 6 psum banks of mm1 output split over 3 tiles; a tile
                # drains as soon as filled while the other two fill/drain, so
                # no engine waits on its own tile's refill.
                RSL = [2, 2, 2]
                LAP = sum(RSL)
                rg = [ppr.tile([128, 512 * n], F32, tag=f"rg{i}", name=f"rg{i}")
                      for i, n in enumerate(RSL)]
                RTILE = []      # lap position -> (tile idx, local slot)
                for i, n in enumerate(RSL):
                    for j in range(n):
                        RTILE.append((i, j))
                dctr = [0]      # global drain counter (engine alternation)

                # software pipeline: at iteration it --
                #   DMA(it) | mm1+drain(it-1) | mm2+chain(it-2) | scatter(it-3)
                T = {}

                def dma_stage(k):
                    ft = fp.tile([37, 2, 4096], FP8E4, tag="ft")
                    nc.sync.dma_start(out=ft[:],
                                      in_=featsT[:, :, k * 4096:(k + 1) * 4096])
                    voh = vp.tile([128, 768 + 64 * W], BF16, tag="voh")
                    nc.sync.dma_start(out=voh[:], in_=vohd[k])
                    T[k] = dict(ft=ft, vt12=voh[:, 0:768], oh=voh[:, 768:])

                def _drain(ti, g_first, nslots):
                    # drain first nslots of ring tile ti; g_first = global mm1
                    # slot index of the tile's first slot (fixes hsb position)
                    cols = 512 * nslots
                    src = rg[ti][:, 0:cols]
                    c0 = (g_first % HSB_SLOTS) * 512
                    dst = hsbring[:, c0:c0 + cols]
                    if dctr[0] % 2 == 0:
                        nc.scalar.activation(out=dst, in_=src, func=AF.Relu)
                    else:
                        nc.vector.tensor_scalar(out=dst, in0=src, scalar1=0.0,
                                                scalar2=None, op0=OP.max)
                    dctr[0] += 1

                def mm1_stage(k, half):
                    d = T[k]
                    for q in range(4 * half, 4 * half + 4):
                        g = 8 * k + q
                        ti, loc = RTILE[g % LAP]
                        nc.tensor.matmul(
                            out=rg[ti][:, 512 * loc:512 * (loc + 1)],
                            lhsT=w1s[:],
                            rhs=d["ft"][:, :, 512 * q:512 * (q + 1)],
                            start=True, stop=True, perf_mode=PM.DoubleRow,
                            skip_group_check=True)
                        if loc == RSL[ti] - 1:
                            _drain(ti, g - loc, RSL[ti])
                        elif g == NSLOT - 1:
                            _drain(ti, g - loc, loc + 1)  # tail flush

                P = {}   # per 2-chunk-pair state (tanh/exp batched: ACT init
                         # overhead halves)

                def mm2_stage(k):
                    p, c = k // 2, k % 2
                    if c == 0:
                        P[p] = dict(bb=pp2.tile([128, 128], F32, tag="bb", name="bb"))
                    bb = P[p]["bb"]
                    base = ((8 * k) % HSB_SLOTS) * 512
                    for tt in range(32):
                        nc.tensor.matmul(
                            out=bb[:, 64 * c + 2 * tt:64 * c + 2 * tt + 2],
                            lhsT=hsbring[:, base + 128 * tt:base + 128 * (tt + 1)],
                            rhs=w2bs[:],
                            start=(tt == 0 and c == 0),
                            stop=(tt == 31 and c == 1),
                            skip_group_check=True)

                def chain_t(p):
                    # tanh for both chunks of pair p
                    bb = P[p]["bb"]
                    th = sp.tile([128, 128], F32, tag="th", name="th")
                    nc.scalar.activation(out=th[:], in_=bb[:], func=AF.Tanh,
                                         bias=b2hs, scale=0.5)
                    # xw cols: [xv_e | xv_o | wv_e | wv_o]
                    P[p].update(th=th, xw=sp.tile([128, 256], F32, tag="xw", name="xw"))

                def chain_a(k):
                    # sig(v) = .5 + .5*tanh(.5*v); x = max(sig*eph, .01)
                    # (sig*eph < 1 always); w = x*eph; vt12 col 10 = eph
                    # last chunks: drains are done; split chains across DVE
                    # and Pool so consecutive chunks' chains run in parallel
                    eng = nc.vector if k == NCHUNK - 2 else nc.gpsimd
                    p, c = k // 2, k % 2
                    vt12v = T[k]["vt12"].rearrange("p (s v) -> p s v", v=12)
                    th = P[p]["th"][:, 64 * c:64 * c + 64]
                    xw = P[p]["xw"]
                    xv = xw[:, 64 * c:64 * c + 64]
                    wv = xw[:, 128 + 64 * c:192 + 64 * c]
                    eph = vt12v[:, :, 10:11]
                    sg = sp.tile([128, 64], F32, tag="sg", name="sg")
                    eng.tensor_scalar(out=sg[:], in0=th, scalar1=0.5,
                                      scalar2=0.5, op0=OP.mult, op1=OP.add)
                    eng.tensor_tensor(
                        out=xv.rearrange("p (s o) -> p s o", o=1),
                        in0=sg[:].rearrange("p (s o) -> p s o", o=1),
                        in1=eph, op=OP.mult)
                    eng.tensor_scalar(out=xv, in0=xv, scalar1=0.01,
                                      scalar2=None, op0=OP.max)
                    eng.tensor_tensor(
                        out=wv.rearrange("p (s o) -> p s o", o=1),
                        in0=xv.rearrange("p (s o) -> p s o", o=1),
                        in1=eph, op=OP.mult)

                def chain_e(p):
                    # exp for both chunks of pair p
                    ew = sp.tile([128, 128], BF16, tag="ewt", name="ewt")
                    nc.scalar.activation(out=ew[:], in_=P[p]["xw"][:, 128:256],
                                         func=AF.Exp)
                    P[p]["ew"] = ew

                def chain_b(k):
                    eng = nc.vector if k in (NCHUNK - 3, NCHUNK - 1) else \
                        nc.gpsimd
                    p, c = k // 2, k % 2
                    d = T[k]
                    vt12v = d["vt12"].rearrange("p (s v) -> p s v", v=12)
                    ewt = P[p]["ew"][:, 64 * c:64 * c + 64]
                    vt5 = vp.tile([128, 5 * 64], BF16, tag="vt5")
                    v5 = vt5[:].rearrange("p (s v) -> p s v", v=5)
                    eng.tensor_tensor(out=v5,
                                      in0=vt12v[:, :, 0:5],
                                      in1=ewt.to_broadcast([128, 64, 5]),
                                      op=OP.mult)
                    d["vt5"] = vt5
                    if c == 1:
                        del P[p]

                def scatter_stage(k):
                    # col j -> sorted tile S = 64k + 32*(j%2) + j//2
                    d = T[k]
                    vt12 = d["vt12"]
                    vt5 = d["vt5"]
                    for j in range(64):
                        S = 64 * k + 32 * (j % 2) + (j // 2)
                        f = _window_start(S)
                        last = (k % 8 == 7 and j >= 62)
                        nc.tensor.matmul(out=scat[0:5, f:f + W],
                                         lhsT=vt5[:, 5 * j:5 * j + 5],
                                         rhs=d["oh"][:, W * j:W * j + W],
                                         start=False, stop=last,
                                         skip_group_check=True)
                        nc.tensor.matmul(out=scat[32:42, f:f + W],
                                         lhsT=vt12[:, 12 * j:12 * j + 10],
                                         rhs=d["oh"][:, W * j:W * j + W],
                                         start=False, stop=last,
                                         skip_group_check=True)
                    if k % 8 == 7:
                        blk = k // 8
                        dst = sc[:, 512 * blk:512 * (blk + 1)]
                        if blk % 2 == 0:
                            nc.vector.tensor_copy(out=dst, in_=scat[:])
                        else:
                            nc.scalar.activation(out=dst, in_=scat[:],
                                                 func=AF.Copy)
                    # free stale per-chunk state
                    del T[k]

                def zero_stage():
                    # re-zero scat bank; deferred one iteration past the sc
                    # copy so its WAR wait cannot head-of-line block PE's mm1s
                    nc.tensor.matmul(out=scat[:], lhsT=zbf[:, 0:48],
                                     rhs=zbf[:], start=True, stop=False,
                                     skip_group_check=True)

                # tile_wait_until slots pin the Tile scheduler to this exact
                # per-engine interleave (ACT: drain, tanh, drain, exp) --
                # left alone it queues exp right after tanh, and the Pool
                # chain latency then stalls the whole ACT stream.
                for it in range(NCHUNK + 7):
                    if it < NCHUNK:
                        with tc.tile_wait_until(it + 0.0):
                            dma_stage(it)
                    if 2 <= it <= NCHUNK + 1:
                        with tc.tile_wait_until(it + 0.1):
                            mm1_stage(it - 2, 0)
                    if 3 <= it <= NCHUNK + 2:
                        with tc.tile_wait_until(it + 0.2):
                            mm2_stage(it - 3)
                    if 4 <= it <= NCHUNK + 3 and (it - 4) % 2 == 0:
                        with tc.tile_wait_until(it + 0.3):
                            chain_t((it - 4) // 2)
                    if 2 <= it <= NCHUNK + 1:
                        with tc.tile_wait_until(it + 0.4):
                            mm1_stage(it - 2, 1)
                    if 4 <= it <= NCHUNK + 3:
                        with tc.tile_wait_until(it + 0.45):
                            chain_a(it - 4)
                    if 5 <= it <= NCHUNK + 4 and (it - 5) % 2 == 0:
                        with tc.tile_wait_until(it + 0.5):
                            chain_e((it - 5) // 2)
                    if 5 <= it <= NCHUNK + 4:
                        with tc.tile_wait_until(it + 0.55):
                            chain_b(it - 5)
                    if 7 <= it <= NCHUNK + 6 and (it - 7) % 8 == 7 and \
                            (it - 7) // 8 < 3:
                        with tc.tile_wait_until(it + 0.58):
                            zero_stage()
                    if 6 <= it <= NCHUNK + 5:
                        with tc.tile_wait_until(it + 0.6):
                            scatter_stage(it - 6)

            # ---- phase B ------------------------------------------------
            # sc rows: 0=sew, 1:5=sewa, 32=cnt, 33:37=sa, 37=ssur, 38:42=ssq
            pps_cm.__exit__(None, None, None)

            with (
                tc.tile_pool(name="ptps", bufs=4, space="PSUM") as ppt,
                tc.tile_pool(name="ptcs", bufs=1, space="PSUM") as pptc,
                tc.tile_pool(name="mmbps", bufs=1, space="PSUM") as ppm,
            ):
                # batched transposes: 4 x [128,48] per PSUM tile, 1 wide copy
                tt = bp.tile([128, 16 * 48], F32, tag="tt")
                scv = sc[:].rearrange("p (c g) -> p g c", g=16)
                for b4 in range(4):
                    pt = ppt.tile([128, 192], F32, tag="pt")
                    for j in range(4):
                        nc.tensor.transpose(out=pt[:, 48 * j:48 * (j + 1)],
                                            in_=scv[:, 4 * b4 + j, :],
                                            identity=ids[0:48, 0:48])
                    if b4 % 2 == 0:
                        nc.vector.tensor_copy(out=tt[:, 192 * b4:192 * (b4 + 1)],
                                              in_=pt[:])
                    else:
                        nc.scalar.activation(out=tt[:, 192 * b4:192 * (b4 + 1)],
                                             in_=pt[:], func=AF.Copy)
                tv = tt[:].rearrange("p (b q) -> p b q", q=48)
                cnt = tv[:, :, 32:33]    # [128,16,1]
                sa = tv[:, :, 33:37]
                ssur = tv[:, :, 37:38]
                ssq = tv[:, :, 38:42]
                sew = tv[:, :, 0:1]
                sewa = tv[:, :, 1:5]

                def wt(tag):
                    return bp.tile([128, 16], F32, tag=tag, name=tag)

                def v3(t):
                    return t[:].rearrange("p (b a) -> p b a", a=1)

                def w4(tag):
                    t = bp.tile([128, 64], F32, tag=tag, name=tag)
                    return t, t[:].rearrange("p (b a) -> p b a", a=4)

                # batched reciprocals: [cntc | den | cm1] -> one reciprocal
                r3 = bp.tile([128, 48], F32, tag="r3")
                nc.vector.tensor_scalar(
                    out=r3[:, 0:16].rearrange("p (b a) -> p b a", a=1),
                    in0=cnt, scalar1=1.0, scalar2=None, op0=OP.max)
                nc.vector.tensor_scalar(
                    out=r3[:, 16:32].rearrange("p (b a) -> p b a", a=1),
                    in0=sew, scalar1=1.0, scalar2=None, op0=OP.max)
                nc.gpsimd.tensor_scalar(
                    out=r3[:, 32:48].rearrange("p (b a) -> p b a", a=1),
                    in0=cnt, scalar1=-1.0, scalar2=1.0, op0=OP.add, op1=OP.max)
                rr = bp.tile([128, 48], F32, tag="rr")
                nc.vector.reciprocal(out=rr[:], in_=r3[:])
                rc = rr[:, 0:16]
                rden = rr[:, 16:32]
                rcm1 = rr[:, 32:48]

                # softmax/aggregate branch (DVE + ACT)
                agr, agrv = w4("agr")
                nc.vector.tensor_tensor(out=agrv, in0=sewa,
                                        in1=rden.to_broadcast([128, 16, 4]),
                                        op=OP.mult)
                es, esv = w4("es")
                nc.scalar.activation(out=es[:], in_=agr[:], func=AF.Exp)
                ssum = wt("ssum")
                nc.vector.tensor_reduce(out=v3(ssum), in_=esv, axis=AX.X,
                                        op=OP.add)
                rssum = wt("rssum")
                nc.vector.reciprocal(out=rssum[:], in_=ssum[:])
                agg, aggv = w4("agg")
                nc.vector.tensor_tensor(out=aggv, in0=esv,
                                        in1=rssum[:].to_broadcast([128, 16, 4]),
                                        op=OP.mult)

                # variance branch (Pool + DVE); mean^2*cnt = mean*sa, and the
                # rcm1 scale commutes past the reduce
                mean, meanv = w4("mean")
                nc.gpsimd.tensor_tensor(out=meanv, in0=sa,
                                        in1=rc.to_broadcast([128, 16, 4]),
                                        op=OP.mult)
                m2, m2v = w4("m2")
                nc.gpsimd.tensor_tensor(out=m2v, in0=meanv, in1=sa,
                                        op=OP.mult)
                varn, varnv = w4("varn")
                nc.vector.tensor_tensor(out=varnv, in0=ssq, in1=m2v,
                                        op=OP.subtract)
                vmr = wt("vmr")
                nc.vector.tensor_reduce(out=v3(vmr), in_=varnv, axis=AX.X,
                                        op=OP.add)
                vm4 = wt("vm4")
                nc.vector.tensor_tensor(out=vm4[:], in0=vmr[:],
                                        in1=rcm1, op=OP.mult)
                # phic = 1 - min(1, 2*vm) = relu(1 - 0.5*vm4)   (vm = vm4/4)
                phic = wt("phic")
                nc.scalar.activation(out=phic[:], in_=vm4[:], func=AF.Relu,
                                     bias=1.0, scale=-0.5)
                coh = wt("coh")
                nc.vector.tensor_scalar(out=coh[:], in0=vm4[:], scalar1=-0.25,
                                        scalar2=1.0, op0=OP.mult, op1=OP.add)
                perr = wt("perr")
                nc.gpsimd.tensor_tensor(out=v3(perr), in0=ssur,
                                        in1=rc.rearrange("p (b a) -> p b a", a=1),
                                        op=OP.mult)
                integ = wt("integ")
                nc.vector.tensor_scalar(out=integ[:], in0=perr[:], scalar1=-1.0,
                                        scalar2=1.0, op0=OP.mult, op1=OP.add)
                nc.vector.tensor_tensor(out=integ[:], in0=integ[:], in1=phic[:],
                                        op=OP.mult)
                szf = wt("szf")
                nc.gpsimd.tensor_scalar(out=v3(szf), in0=cnt, scalar1=0.05,
                                        scalar2=1.0, op0=OP.mult, op1=OP.min)

                # cluster_out columns only need the stats chain: write + DMA
                # them now so the output transfer overlaps the cluster MLP
                oc = bp.tile([128, 144], F32, tag="oc")
                ocv = oc[:, 0:128].rearrange("p (b q) -> p b q", q=8)
                nc.vector.tensor_copy(out=ocv[:, :, 0:4], in_=aggv)
                nc.scalar.activation(
                    out=ocv[:, :, 4:5],
                    in_=phic[:].rearrange("p (b a) -> p b a", a=1),
                    func=AF.Copy)
                nc.vector.tensor_copy(out=ocv[:, :, 5:6],
                                      in_=coh[:].to_broadcast([128, 16, 1]))
                nc.scalar.activation(
                    out=ocv[:, :, 6:7],
                    in_=perr[:].rearrange("p (b a) -> p b a", a=1),
                    func=AF.Copy)
                nc.vector.tensor_copy(out=ocv[:, :, 7:8],
                                      in_=integ[:].to_broadcast([128, 16, 1]))
                nc.sync.dma_start(out=out_all[:, 0:128], in_=oc[:, 0:128])

                # cluster MLP: cft [128, 16 groups x 8 feats] bf16,
                # hc = relu(cft @ v1blkF) via one block-diagonal matmul
                cft = bp.tile([128, 128], BF16, tag="cft")
                cfv = cft[:].rearrange("p (b q) -> p b q", q=8)
                nc.vector.tensor_copy(out=cfv[:, :, 0:4], in_=aggv)
                nc.scalar.activation(
                    out=cfv[:, :, 4:5],
                    in_=phic[:].rearrange("p (b a) -> p b a", a=1),
                    func=AF.Copy)
                nc.vector.tensor_copy(out=cfv[:, :, 5:6],
                                      in_=coh[:].to_broadcast([128, 16, 1]))
                nc.scalar.activation(
                    out=cfv[:, :, 6:7],
                    in_=szf[:].rearrange("p (b a) -> p b a", a=1),
                    func=AF.Copy)
                nc.gpsimd.memset(cfv[:, :, 7:8], 1.0)
                cfT = pptc.tile([128, 128], BF16, tag="cfT")
                nc.tensor.transpose(out=cfT[:], in_=cft[:], identity=idb_c)
                cftSB = bp.tile([128, 128], BF16, tag="cftSB")
                nc.vector.tensor_copy(out=cftSB[:], in_=cfT[:])
                hcp = ppm.tile([128, 512], F32, tag="hcp")
                nc.tensor.matmul(out=hcp[:], lhsT=cftSB[:], rhs=v1blkF,
                                 start=True, stop=True, skip_group_check=True)
                # relu split ACT/DVE; h-dot reduce split DVE/Pool (parallel)
                hcsb = bp.tile([128, 512], BF16, tag="hcsb")
                nc.scalar.activation(out=hcsb[:], in_=hcp[:], func=AF.Relu)
                hv2 = bp.tile([128, 512], BF16, tag="hv2")
                nc.vector.tensor_tensor(out=hv2[:], in0=hcsb[:], in1=v2rep,
                                        op=OP.mult)
                bb2 = wt("bb2")
                nc.vector.tensor_reduce(
                    out=v3(bb2),
                    in_=hv2[:].rearrange("p (b h) -> p b h", h=32),
                    axis=AX.X, op=OP.add)
                basec = wt("basec")
                nc.scalar.activation(out=basec[:], in_=bb2[:], func=AF.Tanh,
                                     bias=c2h, scale=0.5)
                nc.vector.tensor_scalar(out=basec[:], in0=basec[:], scalar1=0.5,
                                        scalar2=0.5, op0=OP.mult, op1=OP.add)

                # impc = max(basec*phic, .01) (min-1 never binds), own DMA
                impc = oc[:, 128:144]
                nc.vector.tensor_tensor(out=impc, in0=basec[:], in1=phic[:],
                                        op=OP.mult)
                nc.vector.tensor_scalar(out=impc, in0=impc, scalar1=0.01,
                                        scalar2=None, op0=OP.max)
                nc.sync.dma_start(out=out_all[:, 128:144], in_=oc[:, 128:144])
    return nc


_NC_CACHE = None


def _get_program():
    global _NC_CACHE
    if _NC_CACHE is None:
        _NC_CACHE = build_program()
    return _NC_CACHE


def _host_prep_core(c, state, arch, energy, phi_local, surprise, seg_ids):
    B0 = int(np.searchsorted(seg_ids, 2048 * c))
    B1 = int(np.searchsorted(seg_ids, 2048 * (c + 1)))
    Nc = B1 - B0
    lseg = (seg_ids[B0:B1] - 2048 * c).astype(np.int64)
    idx = np.full(NPAD, -1, np.int64)
    rel = np.full(NPAD, PADSEG, np.float32)
    cur = 0
    for S in range(NTILES):
        blk = S // TPB
        f = _window_start(S)
        wlo = 512 * blk + f
        whi = wlo + W
        take = min(128, int(np.searchsorted(lseg, whi)) - cur)
        if take > 0:
            assert lseg[cur] >= wlo, f"core {c} tile {S}: behind-lag"
            sl = np.arange(cur, cur + take)
            idx[S * 128:S * 128 + take] = sl
            rel[S * 128:S * 128 + take] = (lseg[sl] - wlo).astype(np.float32)
            cur += take
    assert cur == Nc, f"core {c}: {Nc - cur} cells not scheduled"
    m = idx >= 0

    def g(x):
        out = np.zeros((NPAD,) + x.shape[1:], np.float32)
        out[m] = x[B0:B1][idx[m]]
        return out

    return g(state), g(arch), g(energy), g(phi_local), g(surprise), rel, m


def _swz(x):
    """[NPAD, Q] cell-major -> [NCHUNK, 128, 64*Q] device layout."""
    Q = x.shape[1]
    return np.ascontiguousarray(
        x.reshape(NCHUNK, 2, 32, 128, Q).transpose(0, 3, 2, 1, 4).reshape(
            NCHUNK, 128, 64 * Q))


def kernel(state, arch, energy, phi_local, surprise, seg_ids, n_clusters,
           W1, b1, W2, b2, V1, c1, V2, c2):
    state = np.asarray(state, np.float32)
    arch = np.asarray(arch, np.float32)
    energy = np.asarray(energy, np.float32)
    phi_local = np.asarray(phi_local, np.float32)
    surprise = np.asarray(surprise, np.float32)
    seg_ids = np.asarray(seg_ids)
    W1 = np.asarray(W1, np.float32); b1 = np.asarray(b1, np.float32)
    W2 = np.asarray(W2, np.float32); b2 = np.asarray(b2, np.float32)
    V1 = np.asarray(V1, np.float32); c1 = np.asarray(c1, np.float32)
    V2 = np.asarray(V2, np.float32); c2 = np.asarray(c2, np.float32)

    w1f = np.zeros((74, 128), np.float32)
    w1f[0:36, 0:64] = W1
    w1f[36:72, 64:128] = W1
    w1f[72, 0:64] = b1
    w1f[72, 64:128] = b1
    w1k = np.ascontiguousarray(
        w1f.reshape(2, 37, 128).transpose(1, 0, 2))
    w2f = np.zeros((128, 2), np.float32)
    w2f[0:64, 0] = W2[:, 0]
    w2f[64:128, 1] = W2[:, 0]
    v1p = np.concatenate([V1, c1.reshape(1, 32)], 0)   # [8, 32]
    v1blkF = np.zeros((128, 512), np.float32)
    for gidx in range(16):
        v1blkF[8 * gidx:8 * gidx + 8, 32 * gidx:32 * gidx + 32] = v1p
    cbf = np.zeros((128, 1154), np.float32)
    cbf[:, 0:2] = w2f
    cbf[:, 2:514] = np.tile(V2[:, 0], (128, 16))
    cbf[:, 514:1026] = v1blkF
    cbf[:, 1026:1154] = np.eye(128, dtype=np.float32)
    cf32 = np.zeros((128, 131), np.float32)
    cf32[:, 0] = 0.5 * b2[0]
    cf32[:, 1] = 0.5 * c2[0]
    cf32[:, 2:130] = np.eye(128, dtype=np.float32)
    cf32[:, 130] = 1.0
    consts = dict(
        w1d=w1k.astype(ml_dtypes.float8_e4m3),
        cbfd=cbf.astype(ml_dtypes.bfloat16),
        cf32d=cf32,
    )
    iw = np.arange(W, dtype=np.float32)

    def _prep(c):
        st, ar, en, ph, su, rel, msk = _host_prep_core(
            c, state, arch, energy, phi_local, surprise, seg_ids)
        f36 = np.concatenate([st.T, ar.T], 0)              # [36, NPAD]
        f74 = np.concatenate(
            [f36.reshape(36, NCHUNK, 2, 4096).transpose(2, 0, 1, 3).reshape(
                72, NPAD // 2),
             np.ones((1, NPAD // 2), np.float32),
             np.zeros((1, NPAD // 2), np.float32)], 0)
        featsT = np.ascontiguousarray(
            f74.reshape(2, 37, NPAD // 2).transpose(1, 0, 2)).astype(
                ml_dtypes.float8_e4m3)
        # vt12: [1(mask), a4, sur, a2_4, eph, pad]
        vt12 = np.zeros((NPAD, 12), np.float32)
        vt12[:, 0] = msk
        vt12[:, 1:5] = ar
        vt12[:, 5] = su
        vt12[:, 6:10] = ar * ar
        vt12[:, 10] = en * ph
        oh = (rel[:, None] == iw[None, :]).astype(np.float32)   # [NPAD, W]
        voh = np.concatenate([_swz(vt12), _swz(oh)], axis=2)
        return dict(featsT=featsT,
                    vohd=np.ascontiguousarray(voh).astype(ml_dtypes.bfloat16),
                    **consts)

    from concurrent.futures import ThreadPoolExecutor
    with ThreadPoolExecutor(NCORES) as ex:
        in_maps = list(ex.map(_prep, range(NCORES)))
    nc = _get_program()
    res = run_bass_kernel_spmd(nc, in_maps, list(range(NCORES)))
    global LAST_RESULT
    LAST_RESULT = res
    outs = res.results
    alls = [np.asarray(outs[c]["out_all"]) for c in range(NCORES)]
    couts = [a[:, 0:128].reshape(2048, 8) for a in alls]
    impcs = [a[:, 128:144].reshape(-1) for a in alls]
    cluster_full = np.concatenate(couts, 0).astype(np.float32)
    impc = np.concatenate(impcs, 0).astype(np.float64)

    # organism-level finale on host (exact, f64)
    K = 16384
    counts = np.bincount(seg_ids, minlength=K)
    valid = counts > 0
    n_valid = max(float(valid.sum()), 1.0)
    aggregate = cluster_full[:, 0:4].astype(np.float64)
    phi_c = cluster_full[:, 4].astype(np.float64)
    coh = cluster_full[:, 5].astype(np.float64)
    iv = np.where(valid, impc, -np.inf)
    e = np.exp(iv - iv.max())
    wc = e / e.sum()
    ga = (wc[:, None] * aggregate).sum(0)
    eg = np.exp(ga - ga.max())
    global_arch = (eg / eg.sum()).astype(np.float32)
    avg_phi = (phi_c * valid).sum() / n_valid
    spec = np.argmax(aggregate, axis=1)
    present = np.zeros(4, bool)
    for a in range(4):
        present[a] = np.any(valid & (spec == a))
    unique = float(present.sum())
    phi_global = min(1.0, avg_phi * (0.5 + 0.5 * unique / 4.0))
    vert = (coh * valid).sum() / n_valid
    self_model = np.array([*global_arch, phi_global, vert], np.float32)
    return np.concatenate([cluster_full.reshape(-1), self_model]).astype(np.float32)


# revision 41
# speedup vs baseline: 1.2100x; 1.0097x over previous
"""Trainium2 Bass kernel for nn_BottomUpIntegrator (gnn_message_passing).

Sharding: cells split at cluster boundaries across 8 cores (2048 clusters
each). Per-core segmented reductions via one-hot scatter matmuls accumulating
into a single persistent PSUM bank with a core-invariant static window
schedule (W=8).

Phase A (per 8192-cell chunk, software-pipelined):
  DMA: feats fp8(e4m3) [37,2,4096] k-tiled for DoubleRow (bias folded as
       ones-row), host-packed bf16 [1|a|sur|a^2|eph] columns + one-hot window
       rows, one merged DMA.
  PE:  mm1 in fp8e4m3 DoubleRow (2 k-tiles of 37 rows, 0.5 cyc/row), mm2
       (base logits, hs-as-weights trick), scatter (vt12 host rows -> PSUM
       parts 32:42, ew rows -> parts 0:5).
  mm1 PSUM: ring of 2x[128,1536] tiles (6 banks); drains of 1536 cols
       (relu + fp32->bf16) alternate ACT/DVE into a 24-slot SBUF hsb ring,
       amortizing per-instruction access-latency overhead.
  ACT: tanh (sigmoid = .5+.5 tanh(v/2)), exp(w).
  Pool (SBUF-only): sigmoid affine, clip chain, ew*[1,a] scatter rows.
Phase B: strided transposes (batched 4-per-PSUM-tile, 4 wide copies),
  stats chain distributed across DVE/Pool/ACT, cluster MLP via one
  block-diagonal matmul (clusters stay on partitions; V1 bias via
  ones-feature, V2 dot via bf16 elementwise+reduce).
Organism-level finale (valid mask, argmax/present, softmax weights) runs on
host in f64 from per-cluster outputs + impc (host knows exact counts).
"""
import numpy as np
import ml_dtypes

import json as _json

from concourse import bass, mybir
from concourse import bass2jax as _b2j
from concourse import bass_utils as _bu
from concourse.tile import TileContext
from concourse.bass_utils import run_bass_kernel_spmd

_orig_compile = _bu.compile_bir_kernel


def _split_waits_compile(bir_json, tmpdir, neff_name="file.neff"):
    """Walrus lowers at most ONE semaphore wait per TPB instruction struct.
    Tile emits several. Hoist extras onto injected same-engine EventSemaphore
    wait instructions immediately before the owner (semantically identical:
    engines execute in program order)."""
    d = _json.loads(bir_json)
    cnt = 0
    for fn in d["functions"]:
        for blk in fn["blocks"]:
            newlist = []
            for ins in blk["instructions"]:
                si = ins.get("sync_info")
                waits = si.get("on_wait", []) if si else []
                if si and len(waits) > 1 and ins.get("opcode") not in (
                        "EventSemaphore",):
                    for w_i, w in enumerate(waits[:-1]):
                        cnt += 1
                        newlist.append({
                            "debug": ins.get("debug", 0),
                            "engine": ins["engine"],
                            "ins": [], "outs": [],
                            "name": f"{ins['name']}-wsplit{w_i}",
                            "opcode": "EventSemaphore",
                            "sync_info": {"on_update": [], "on_wait": [w]},
                        })
                    si["on_wait"] = [waits[-1]]
                newlist.append(ins)
            blk["instructions"] = newlist
    print(f"[wait-split] hoisted {cnt} extra waits")
    return _orig_compile(_json.dumps(d).encode(), tmpdir, neff_name=neff_name)


_bu.compile_bir_kernel = _split_waits_compile
_b2j.compile_bir_kernel = _split_waits_compile

F32 = mybir.dt.float32
BF16 = mybir.dt.bfloat16
FP8E4 = mybir.dt.float8e4
AF = mybir.ActivationFunctionType
OP = mybir.AluOpType
AX = mybir.AxisListType
PM = mybir.MatmulPerfMode

NCORES = 8
KLOC = 2048            # clusters per core
NPAD = 262144          # padded cells per core
CHUNK = 8192           # cells per chunk
NCHUNK = NPAD // CHUNK # 32
W = 8                  # onehot window width (clusters)
MARGIN = 2             # window start = clip(s - MARGIN, 0, 512 - W)
NTILES = NPAD // 128   # 2048 scatter tiles per core
TPB = NTILES // 4      # tiles per 512-cluster block
PADSEG = 1.0e9
NSLOT = NPAD // 2 // 512   # 256 global mm1 psum 512-col slots
HSB_SLOTS = 24             # hsb ring capacity in 512-col slots (3 chunks)

# scatter stationary rows, accumulated in one scat bank [48, 512]:
#  parts 0:5   <- vt5  (device): [ew, ew*a4]
#  parts 32:42 <- vt12 (host):   [1, a4, sur, a2_4]  (lhsT cols 0:10)
# vt12 (host) col layout: [1, a4, sur, a2_4, eph, pad]


def _window_start(S):
    s = S % TPB
    return int(np.clip(s - MARGIN, 0, 512 - W))


def build_program():
    nc = bass.Bass(trn_type="TRN2", use_seq_codegen=True)
    featsT = nc.dram_tensor("featsT", [37, 2, NPAD // 2], FP8E4,
                            kind="ExternalInput")
    vohd = nc.dram_tensor("vohd", [NCHUNK, 128, 768 + 64 * W], BF16,
                          kind="ExternalInput")
    w1d = nc.dram_tensor("w1d", [37, 2, 128], FP8E4, kind="ExternalInput")
    cbfd = nc.dram_tensor("cbfd", [128, 1154], BF16, kind="ExternalInput")
    cf32d = nc.dram_tensor("cf32d", [128, 131], F32, kind="ExternalInput")
    out_all = nc.dram_tensor("out_all", [128, 144], F32, kind="ExternalOutput")

    with TileContext(nc) as tc:
        with (
            tc.tile_pool(name="const", bufs=1) as cp,
            tc.tile_pool(name="feats", bufs=4) as fp,
            tc.tile_pool(name="small", bufs=4) as sp,
            tc.tile_pool(name="scatv", bufs=7) as vp,
            tc.tile_pool(name="ph_b", bufs=1) as bp,
        ):
            # ---- constants ----------------------------------------------
            w1s = cp.tile([37, 2, 128], FP8E4, tag="w1s")
            nc.sync.dma_start(out=w1s[:], in_=w1d[:])
            cbf = cp.tile([128, 1154], BF16, tag="cbf")
            nc.sync.dma_start(out=cbf[:], in_=cbfd[:])
            cf32 = cp.tile([128, 131], F32, tag="cf32")
            nc.sync.dma_start(out=cf32[:], in_=cf32d[:])
            w2bs = cbf[:, 0:2]
            v2rep = cbf[:, 2:514]
            v1blkF = cbf[:, 514:1026]
            idb_c = cbf[:, 1026:1154]
            b2hs = cf32[:, 0:1]
            c2h = cf32[:, 1:2]
            ids = cf32[:, 2:130]
            zbf = cp.tile([128, 512], BF16, tag="zbf")
            nc.vector.memset(zbf[:], 0.0)
            # hsb ring: 24 x 512-col slots of relu'd mm1 output (bf16)
            hsbring = cp.tile([128, HSB_SLOTS * 512], BF16, tag="hsbring")

            # Pre-touch DMA-loaded constants on their consuming engines so no
            # later compute instruction needs a second (DMA) semaphore wait.
            scra = cp.tile([128, 2], F32, tag="scra")
            nc.scalar.activation(out=scra[:, 0:1], in_=b2hs, func=AF.Copy)
            nc.scalar.activation(out=scra[:, 1:2], in_=c2h, func=AF.Copy)

            # persistent scatter accumulator: 1 PSUM bank [48, 512]
            pps_cm = tc.tile_pool(name="scatps", bufs=1, space="PSUM")
            pps = pps_cm.__enter__()
            scat = pps.tile([48, 512], F32, tag="scat", name="scat")
            # PE touch of PE-consumed consts (rides on Ldweights; overwritten
            # by the zeroing matmul below).
            nc.tensor.matmul(out=scat[0:1, 0:1], lhsT=ids[0:1, 0:1],
                             rhs=ids[0:1, 0:1], start=True, stop=True,
                             skip_group_check=True)
            nc.tensor.matmul(out=scat[0:1, 0:2], lhsT=w1s[0:1, 0, 0:1],
                             rhs=w1s[0:1, 0, 0:2], start=True, stop=True,
                             skip_group_check=True)
            nc.tensor.matmul(out=scat[0:2, 0:2], lhsT=w2bs[0:1, :],
                             rhs=w2bs[0:1, :], start=True, stop=True,
                             skip_group_check=True)
            nc.tensor.matmul(out=scat[:], lhsT=zbf[:, 0:48], rhs=zbf[:],
                             start=True, stop=False, skip_group_check=True)
            sc = bp.tile([48, 2048], F32, tag="sc")

            # ---- phase A ------------------------------------------------
            with (
                tc.tile_pool(name="ringps", bufs=1, space="PSUM") as ppr,
                tc.tile_pool(name="mm2ps", bufs=1, space="PSUM") as pp2,
            ):
                # ring: 6 psum banks of mm1 output split over 3 tiles; a tile
                # drains as soon as filled while the other two fill/drain, so
                # no engine waits on its own tile's refill.
                RSL = [2, 2, 2]
                LAP = sum(RSL)
                rg = [ppr.tile([128, 512 * n], F32, tag=f"rg{i}", name=f"rg{i}")
                      for i, n in enumerate(RSL)]
                RTILE = []      # lap position -> (tile idx, local slot)
                for i, n in enumerate(RSL):
                    for j in range(n):
                        RTILE.append((i, j))
                dctr = [0]      # global drain counter (engine alternation)

                # software pipeline: at iteration it --
                #   DMA(it) | mm1+drain(it-1) | mm2+chain(it-2) | scatter(it-3)
                T = {}

                def dma_stage(k):
                    ft = fp.tile([37, 2, 4096], FP8E4, tag="ft")
                    nc.sync.dma_start(out=ft[:],
                                      in_=featsT[:, :, k * 4096:(k + 1) * 4096])
                    voh = vp.tile([128, 768 + 64 * W], BF16, tag="voh")
                    nc.sync.dma_start(out=voh[:], in_=vohd[k])
                    T[k] = dict(ft=ft, vt12=voh[:, 0:768], oh=voh[:, 768:])

                def _drain(ti, g_first, nslots):
                    # drain first nslots of ring tile ti; g_first = global mm1
                    # slot index of the tile's first slot (fixes hsb position)
                    cols = 512 * nslots
                    src = rg[ti][:, 0:cols]
                    c0 = (g_first % HSB_SLOTS) * 512
                    dst = hsbring[:, c0:c0 + cols]
                    if dctr[0] % 2 == 0:
                        nc.scalar.activation(out=dst, in_=src, func=AF.Relu)
                    else:
                        nc.vector.tensor_scalar(out=dst, in0=src, scalar1=0.0,
                                                scalar2=None, op0=OP.max)
                    dctr[0] += 1

                def mm1_stage(k, half):
                    d = T[k]
                    for q in range(4 * half, 4 * half + 4):
                        g = 8 * k + q
                        ti, loc = RTILE[g % LAP]
                        nc.tensor.matmul(
                            out=rg[ti][:, 512 * loc:512 * (loc + 1)],
                            lhsT=w1s[:],
                            rhs=d["ft"][:, :, 512 * q:512 * (q + 1)],
                            start=True, stop=True, perf_mode=PM.DoubleRow,
                            skip_group_check=True)
                        if loc == RSL[ti] - 1:
                            _drain(ti, g - loc, RSL[ti])
                        elif g == NSLOT - 1:
                            _drain(ti, g - loc, loc + 1)  # tail flush

                P = {}   # per 2-chunk-pair state (tanh/exp batched: ACT init
                         # overhead halves)

                def mm2_stage(k):
                    p, c = k // 2, k % 2
                    if c == 0:
                        P[p] = dict(bb=pp2.tile([128, 128], F32, tag="bb", name="bb"))
                    bb = P[p]["bb"]
                    base = ((8 * k) % HSB_SLOTS) * 512
                    for tt in range(32):
                        nc.tensor.matmul(
                            out=bb[:, 64 * c + 2 * tt:64 * c + 2 * tt + 2],
                            lhsT=hsbring[:, base + 128 * tt:base + 128 * (tt + 1)],
                            rhs=w2bs[:],
                            start=(tt == 0 and c == 0),
                            stop=(tt == 31 and c == 1),
                            skip_group_check=True)

                def chain_t(p, half=None):
                    # tanh for pair p (or one chunk of it in the tail, where
                    # waiting for the pair's second mm2 would serialize)
                    bb = P[p]["bb"]
                    if half is None or half == 0:
                        th = sp.tile([128, 128], F32, tag="th", name="th")
                        # xw cols: [xv_e | xv_o | wv_e | wv_o]
                        P[p].update(th=th,
                                    xw=sp.tile([128, 256], F32, tag="xw",
                                               name="xw"))
                    th = P[p]["th"]
                    if half is None:
                        nc.scalar.activation(out=th[:], in_=bb[:],
                                             func=AF.Tanh, bias=b2hs, scale=0.5)
                    else:
                        sl = slice(64 * half, 64 * half + 64)
                        nc.scalar.activation(out=th[:, sl], in_=bb[:, sl],
                                             func=AF.Tanh, bias=b2hs, scale=0.5)

                def chain_a(k):
                    # sig(v) = .5 + .5*tanh(.5*v); x = max(sig*eph, .01)
                    # (sig*eph < 1 always); w = x*eph; vt12 col 10 = eph
                    # last chunks: drains are done; split chains across DVE
                    # and Pool so consecutive chunks' chains run in parallel
                    eng = nc.vector if k == NCHUNK - 1 else nc.gpsimd
                    p, c = k // 2, k % 2
                    vt12v = T[k]["vt12"].rearrange("p (s v) -> p s v", v=12)
                    th = P[p]["th"][:, 64 * c:64 * c + 64]
                    xw = P[p]["xw"]
                    xv = xw[:, 64 * c:64 * c + 64]
                    wv = xw[:, 128 + 64 * c:192 + 64 * c]
                    eph = vt12v[:, :, 10:11]
                    sg = sp.tile([128, 64], F32, tag="sg", name="sg")
                    eng.tensor_scalar(out=sg[:], in0=th, scalar1=0.5,
                                      scalar2=0.5, op0=OP.mult, op1=OP.add)
                    eng.tensor_tensor(
                        out=xv.rearrange("p (s o) -> p s o", o=1),
                        in0=sg[:].rearrange("p (s o) -> p s o", o=1),
                        in1=eph, op=OP.mult)
                    eng.tensor_scalar(out=xv, in0=xv, scalar1=0.01,
                                      scalar2=None, op0=OP.max)
                    eng.tensor_tensor(
                        out=wv.rearrange("p (s o) -> p s o", o=1),
                        in0=xv.rearrange("p (s o) -> p s o", o=1),
                        in1=eph, op=OP.mult)

                def chain_e(p, half=None):
                    # exp for pair p (or one chunk in the tail)
                    if half is None or half == 0:
                        P[p]["ew"] = sp.tile([128, 128], BF16, tag="ewt",
                                             name="ewt")
                    ew = P[p]["ew"]
                    if half is None:
                        nc.scalar.activation(out=ew[:],
                                             in_=P[p]["xw"][:, 128:256],
                                             func=AF.Exp)
                    else:
                        nc.scalar.activation(
                            out=ew[:, 64 * half:64 * half + 64],
                            in_=P[p]["xw"][:, 128 + 64 * half:192 + 64 * half],
                            func=AF.Exp)

                def chain_b(k):
                    eng = nc.vector if k in (NCHUNK - 3, NCHUNK - 1) else \
                        nc.gpsimd
                    p, c = k // 2, k % 2
                    d = T[k]
                    vt12v = d["vt12"].rearrange("p (s v) -> p s v", v=12)
                    ewt = P[p]["ew"][:, 64 * c:64 * c + 64]
                    vt5 = vp.tile([128, 5 * 64], BF16, tag="vt5")
                    v5 = vt5[:].rearrange("p (s v) -> p s v", v=5)
                    eng.tensor_tensor(out=v5,
                                      in0=vt12v[:, :, 0:5],
                                      in1=ewt.to_broadcast([128, 64, 5]),
                                      op=OP.mult)
                    d["vt5"] = vt5
                    if c == 1:
                        del P[p]

                def scatter_stage(k):
                    # col j -> sorted tile S = 64k + 32*(j%2) + j//2
                    d = T[k]
                    vt12 = d["vt12"]
                    vt5 = d["vt5"]
                    for j in range(64):
                        S = 64 * k + 32 * (j % 2) + (j // 2)
                        f = _window_start(S)
                        last = (k % 8 == 7 and j >= 62)
                        nc.tensor.matmul(out=scat[0:5, f:f + W],
                                         lhsT=vt5[:, 5 * j:5 * j + 5],
                                         rhs=d["oh"][:, W * j:W * j + W],
                                         start=False, stop=last,
                                         skip_group_check=True)
                        nc.tensor.matmul(out=scat[32:42, f:f + W],
                                         lhsT=vt12[:, 12 * j:12 * j + 10],
                                         rhs=d["oh"][:, W * j:W * j + W],
                                         start=False, stop=last,
                                         skip_group_check=True)
                    if k % 8 == 7:
                        blk = k // 8
                        dst = sc[:, 512 * blk:512 * (blk + 1)]
                        if blk == 3:
                            # tail: split halves across ACT+DVE (both idle)
                            nc.vector.tensor_copy(out=dst[:, 0:256],
                                                  in_=scat[:, 0:256])
                            nc.scalar.activation(out=dst[:, 256:512],
                                                 in_=scat[:, 256:512],
                                                 func=AF.Copy)
                        elif blk % 2 == 0:
                            nc.vector.tensor_copy(out=dst, in_=scat[:])
                        else:
                            nc.scalar.activation(out=dst, in_=scat[:],
                                                 func=AF.Copy)
                    # free stale per-chunk state
                    del T[k]

                def zero_stage():
                    # re-zero scat bank; deferred one iteration past the sc
                    # copy so its WAR wait cannot head-of-line block PE's mm1s
                    nc.tensor.matmul(out=scat[:], lhsT=zbf[:, 0:48],
                                     rhs=zbf[:], start=True, stop=False,
                                     skip_group_check=True)

                # tile_wait_until slots pin the Tile scheduler to this exact
                # per-engine interleave (ACT: drain, tanh, drain, exp) --
                # left alone it queues exp right after tanh, and the Pool
                # chain latency then stalls the whole ACT stream.
                for it in range(NCHUNK + 7):
                    if it < NCHUNK:
                        with tc.tile_wait_until(it + 0.0):
                            dma_stage(it)
                    if 2 <= it <= NCHUNK + 1:
                        with tc.tile_wait_until(it + 0.1):
                            mm1_stage(it - 2, 0)
                    if 3 <= it <= NCHUNK + 2:
                        with tc.tile_wait_until(it + 0.2):
                            mm2_stage(it - 3)
                    if 4 <= it <= NCHUNK + 1 and (it - 4) % 2 == 0:
                        with tc.tile_wait_until(it + 0.3):
                            chain_t((it - 4) // 2)
                    if it in (NCHUNK + 1, NCHUNK + 2):
                        with tc.tile_wait_until(it + 0.25):
                            chain_t(NCHUNK // 2 - 1, half=it - NCHUNK - 1)
                    if 2 <= it <= NCHUNK + 1:
                        with tc.tile_wait_until(it + 0.4):
                            mm1_stage(it - 2, 1)
                    if 4 <= it <= NCHUNK + 1:
                        with tc.tile_wait_until(it + 0.45):
                            chain_a(it - 4)
                    if it in (NCHUNK + 1, NCHUNK + 2):
                        with tc.tile_wait_until(it + 0.45):
                            chain_a(it - 3)
                    if 5 <= it <= NCHUNK + 2 and (it - 5) % 2 == 0:
                        with tc.tile_wait_until(it + 0.5):
                            chain_e((it - 5) // 2)
                    if it in (NCHUNK + 1, NCHUNK + 2):
                        with tc.tile_wait_until(it + 0.5):
                            chain_e(NCHUNK // 2 - 1, half=it - NCHUNK - 1)
                    if 5 <= it <= NCHUNK + 2:
                        with tc.tile_wait_until(it + 0.55):
                            chain_b(it - 5)
                    if it == NCHUNK + 1:
                        with tc.tile_wait_until(it + 0.56):
                            chain_b(NCHUNK - 2)
                    if it == NCHUNK + 2:
                        with tc.tile_wait_until(it + 0.56):
                            chain_b(NCHUNK - 1)
                    if 7 <= it <= NCHUNK + 6 and (it - 7) % 8 == 7 and \
                            (it - 7) // 8 < 3:
                        with tc.tile_wait_until(it + 0.58):
                            zero_stage()
                    if 6 <= it <= NCHUNK + 3:
                        with tc.tile_wait_until(it + 0.6):
                            scatter_stage(it - 6)
                    if it == NCHUNK + 3:
                        with tc.tile_wait_until(it + 0.65):
                            scatter_stage(NCHUNK - 2)
                        with tc.tile_wait_until(it + 0.7):
                            scatter_stage(NCHUNK - 1)

            # ---- phase B ------------------------------------------------
            # sc rows: 0=sew, 1:5=sewa, 32=cnt, 33:37=sa, 37=ssur, 38:42=ssq
            pps_cm.__exit__(None, None, None)

            with (
                tc.tile_pool(name="ptps", bufs=4, space="PSUM") as ppt,
                tc.tile_pool(name="ptcs", bufs=1, space="PSUM") as pptc,
                tc.tile_pool(name="mmbps", bufs=1, space="PSUM") as ppm,
            ):
                # batched transposes: 4 x [128,48] per PSUM tile, 1 wide copy
                tt = bp.tile([128, 16 * 48], F32, tag="tt")
                scv = sc[:].rearrange("p (c g) -> p g c", g=16)
                with tc.tile_wait_until(40.1):
                    for b4 in range(4):
                        pt = ppt.tile([128, 192], F32, tag="pt", name="pt")
                        for j in range(4):
                            nc.tensor.transpose(out=pt[:, 48 * j:48 * (j + 1)],
                                                in_=scv[:, 4 * b4 + j, :],
                                                identity=ids[0:48, 0:48])
                        if b4 % 2 == 0:
                            nc.vector.tensor_copy(
                                out=tt[:, 192 * b4:192 * (b4 + 1)], in_=pt[:])
                        else:
                            nc.scalar.activation(
                                out=tt[:, 192 * b4:192 * (b4 + 1)], in_=pt[:],
                                func=AF.Copy)
                tv = tt[:].rearrange("p (b q) -> p b q", q=48)
                cnt = tv[:, :, 32:33]    # [128,16,1]
                sa = tv[:, :, 33:37]
                ssur = tv[:, :, 37:38]
                ssq = tv[:, :, 38:42]
                sew = tv[:, :, 0:1]
                sewa = tv[:, :, 1:5]

                def wt(tag):
                    return bp.tile([128, 16], F32, tag=tag, name=tag)

                def v3(t):
                    return t[:].rearrange("p (b a) -> p b a", a=1)

                def w4(tag):
                    t = bp.tile([128, 64], F32, tag=tag, name=tag)
                    return t, t[:].rearrange("p (b a) -> p b a", a=4)

                # batched reciprocals: [cntc | den | cm1] -> one reciprocal
                r3 = bp.tile([128, 48], F32, tag="r3")
                rr = bp.tile([128, 48], F32, tag="rr")
                szf = wt("szf")
                with tc.tile_wait_until(40.2):
                    nc.vector.tensor_scalar(
                        out=r3[:, 0:16].rearrange("p (b a) -> p b a", a=1),
                        in0=cnt, scalar1=1.0, scalar2=None, op0=OP.max)
                    nc.vector.tensor_scalar(
                        out=r3[:, 16:32].rearrange("p (b a) -> p b a", a=1),
                        in0=sew, scalar1=1.0, scalar2=None, op0=OP.max)
                    nc.gpsimd.tensor_scalar(
                        out=r3[:, 32:48].rearrange("p (b a) -> p b a", a=1),
                        in0=cnt, scalar1=-1.0, scalar2=1.0, op0=OP.add,
                        op1=OP.max)
                    nc.vector.reciprocal(out=rr[:], in_=r3[:])
                    nc.gpsimd.tensor_scalar(out=v3(szf), in0=cnt, scalar1=0.05,
                                            scalar2=1.0, op0=OP.mult,
                                            op1=OP.min)
                rc = rr[:, 0:16]
                rden = rr[:, 16:32]
                rcm1 = rr[:, 32:48]

                # variance chain all-DVE (critical path: no cross-engine
                # hops); softmax/aggregate interleaved; perr on Pool
                agr, agrv = w4("agr")
                es, esv = w4("es")
                ssum = wt("ssum")
                rssum = wt("rssum")
                agg, aggv = w4("agg")
                mean, meanv = w4("mean")
                m2, m2v = w4("m2")
                varn, varnv = w4("varn")
                vmr = wt("vmr")
                vm4 = wt("vm4")
                phic = wt("phic")
                coh = wt("coh")
                perr = wt("perr")
                integ = wt("integ")
                oc = bp.tile([128, 144], F32, tag="oc")
                ocv = oc[:, 0:128].rearrange("p (b q) -> p b q", q=8)
                cft = bp.tile([128, 128], BF16, tag="cft")
                cfv = cft[:].rearrange("p (b q) -> p b q", q=8)
                phic_f = ocv[:, :, 4:5]       # f32 phic (output + integ/impc)
                with tc.tile_wait_until(40.3):
                    nc.vector.tensor_tensor(out=meanv, in0=sa,
                                            in1=rc.to_broadcast([128, 16, 4]),
                                            op=OP.mult)
                    nc.vector.tensor_tensor(out=agrv, in0=sewa,
                                            in1=rden.to_broadcast([128, 16, 4]),
                                            op=OP.mult)
                    nc.scalar.activation(out=es[:], in_=agr[:], func=AF.Exp)
                    nc.vector.tensor_tensor(out=m2v, in0=meanv, in1=sa,
                                            op=OP.mult)
                    nc.vector.tensor_tensor(out=varnv, in0=ssq, in1=m2v,
                                            op=OP.subtract)
                    nc.vector.tensor_reduce(out=v3(vmr), in_=varnv, axis=AX.X,
                                            op=OP.add)
                    nc.vector.tensor_tensor(out=vm4[:], in0=vmr[:],
                                            in1=rcm1, op=OP.mult)
                    # phic = 1 - min(1, 2*vm) = relu(1 - .5*vm4)  (vm = vm4/4)
                    # written twice at source precision: bf16 into the MLP
                    # feature tile, f32 into the output tile
                    nc.scalar.activation(
                        out=cfv[:, :, 4:5],
                        in_=vm4[:].rearrange("p (b a) -> p b a", a=1),
                        func=AF.Relu, bias=1.0, scale=-0.5)
                    nc.scalar.activation(
                        out=phic_f,
                        in_=vm4[:].rearrange("p (b a) -> p b a", a=1),
                        func=AF.Relu, bias=1.0, scale=-0.5)
                    nc.vector.tensor_reduce(out=v3(ssum), in_=esv, axis=AX.X,
                                            op=OP.add)
                    nc.vector.reciprocal(out=rssum[:], in_=ssum[:])
                    nc.vector.tensor_tensor(
                        out=ocv[:, :, 0:4], in0=esv,
                        in1=rssum[:].to_broadcast([128, 16, 4]), op=OP.mult)
                    nc.gpsimd.tensor_tensor(
                        out=ocv[:, :, 6:7], in0=ssur,
                        in1=rc.rearrange("p (b a) -> p b a", a=1), op=OP.mult)
                    nc.vector.tensor_scalar(
                        out=ocv[:, :, 5:6].rearrange("p b a -> p (b a)"),
                        in0=vm4[:], scalar1=-0.25, scalar2=1.0,
                        op0=OP.mult, op1=OP.add)
                    nc.gpsimd.tensor_scalar(
                        out=v3(integ),
                        in0=ocv[:, :, 6:7], scalar1=-1.0, scalar2=1.0,
                        op0=OP.mult, op1=OP.add)
                    nc.gpsimd.tensor_tensor(out=ocv[:, :, 7:8],
                                            in0=v3(integ),
                                            in1=phic_f, op=OP.mult)

                # cluster MLP input cft [128, 16 groups x 8 feats] bf16
                with tc.tile_wait_until(40.4):
                    nc.gpsimd.memset(cfv[:, :, 7:8], 1.0)
                    nc.scalar.activation(
                        out=cfv[:, :, 6:7],
                        in_=szf[:].rearrange("p (b a) -> p b a", a=1),
                        func=AF.Copy)
                    nc.vector.tensor_copy(out=cfv[:, :, 0:4],
                                          in_=ocv[:, :, 0:4])
                    nc.vector.tensor_copy(out=cfv[:, :, 5:6],
                                          in_=ocv[:, :, 5:6])
                cfT = pptc.tile([128, 128], BF16, tag="cfT")
                cftSB = bp.tile([128, 128], BF16, tag="cftSB")
                hcp = ppm.tile([128, 512], F32, tag="hcp")
                hv2 = bp.tile([128, 512], BF16, tag="hv2")
                bb2 = wt("bb2")
                basec = wt("basec")
                with tc.tile_wait_until(40.5):
                    nc.tensor.transpose(out=cfT[:], in_=cft[:], identity=idb_c)
                    nc.vector.tensor_copy(out=cftSB[:], in_=cfT[:])
                    nc.tensor.matmul(out=hcp[:], lhsT=cftSB[:], rhs=v1blkF,
                                     start=True, stop=True,
                                     skip_group_check=True)
                    # hv2 = relu(hc) * V2 in one fused op straight from psum
                    nc.vector.scalar_tensor_tensor(
                        out=hv2[:], in0=hcp[:], scalar=0.0, in1=v2rep,
                        op0=OP.max, op1=OP.mult)
                    nc.vector.tensor_reduce(
                        out=v3(bb2),
                        in_=hv2[:].rearrange("p (b h) -> p b h", h=32),
                        axis=AX.X, op=OP.add)
                    nc.scalar.activation(out=basec[:], in_=bb2[:], func=AF.Tanh,
                                         bias=c2h, scale=0.5)

                # cluster_out cols were written in place: DMA overlaps MLP
                with tc.tile_wait_until(40.45):
                    nc.sync.dma_start(out=out_all[:, 0:128], in_=oc[:, 0:128])

                impc = oc[:, 128:144]
                with tc.tile_wait_until(40.6):
                    # impc = max(basec*phic, .01): sig affine folded in; the
                    # min-1 clip never binds (basec < 1, phic <= 1)
                    nc.vector.tensor_scalar(out=basec[:], in0=basec[:],
                                            scalar1=0.5, scalar2=0.5,
                                            op0=OP.mult, op1=OP.add)
                    nc.vector.tensor_tensor(out=v3(impc), in0=v3(basec),
                                            in1=phic_f, op=OP.mult)
                    nc.vector.tensor_scalar(out=impc, in0=impc, scalar1=0.01,
                                            scalar2=None, op0=OP.max)
                    nc.sync.dma_start(out=out_all[:, 128:144],
                                      in_=oc[:, 128:144])
    return nc


_NC_CACHE = None


def _get_program():
    global _NC_CACHE
    if _NC_CACHE is None:
        _NC_CACHE = build_program()
    return _NC_CACHE


def _host_prep_core(c, state, arch, energy, phi_local, surprise, seg_ids):
    B0 = int(np.searchsorted(seg_ids, 2048 * c))
    B1 = int(np.searchsorted(seg_ids, 2048 * (c + 1)))
    Nc = B1 - B0
    lseg = (seg_ids[B0:B1] - 2048 * c).astype(np.int64)
    idx = np.full(NPAD, -1, np.int64)
    rel = np.full(NPAD, PADSEG, np.float32)
    cur = 0
    for S in range(NTILES):
        blk = S // TPB
        f = _window_start(S)
        wlo = 512 * blk + f
        whi = wlo + W
        take = min(128, int(np.searchsorted(lseg, whi)) - cur)
        if take > 0:
            assert lseg[cur] >= wlo, f"core {c} tile {S}: behind-lag"
            sl = np.arange(cur, cur + take)
            idx[S * 128:S * 128 + take] = sl
            rel[S * 128:S * 128 + take] = (lseg[sl] - wlo).astype(np.float32)
            cur += take
    assert cur == Nc, f"core {c}: {Nc - cur} cells not scheduled"
    m = idx >= 0

    def g(x):
        out = np.zeros((NPAD,) + x.shape[1:], np.float32)
        out[m] = x[B0:B1][idx[m]]
        return out

    return g(state), g(arch), g(energy), g(phi_local), g(surprise), rel, m


def _swz(x):
    """[NPAD, Q] cell-major -> [NCHUNK, 128, 64*Q] device layout."""
    Q = x.shape[1]
    return np.ascontiguousarray(
        x.reshape(NCHUNK, 2, 32, 128, Q).transpose(0, 3, 2, 1, 4).reshape(
            NCHUNK, 128, 64 * Q))


def kernel(state, arch, energy, phi_local, surprise, seg_ids, n_clusters,
           W1, b1, W2, b2, V1, c1, V2, c2):
    state = np.asarray(state, np.float32)
    arch = np.asarray(arch, np.float32)
    energy = np.asarray(energy, np.float32)
    phi_local = np.asarray(phi_local, np.float32)
    surprise = np.asarray(surprise, np.float32)
    seg_ids = np.asarray(seg_ids)
    W1 = np.asarray(W1, np.float32); b1 = np.asarray(b1, np.float32)
    W2 = np.asarray(W2, np.float32); b2 = np.asarray(b2, np.float32)
    V1 = np.asarray(V1, np.float32); c1 = np.asarray(c1, np.float32)
    V2 = np.asarray(V2, np.float32); c2 = np.asarray(c2, np.float32)

    w1f = np.zeros((74, 128), np.float32)
    w1f[0:36, 0:64] = W1
    w1f[36:72, 64:128] = W1
    w1f[72, 0:64] = b1
    w1f[72, 64:128] = b1
    w1k = np.ascontiguousarray(
        w1f.reshape(2, 37, 128).transpose(1, 0, 2))
    w2f = np.zeros((128, 2), np.float32)
    w2f[0:64, 0] = W2[:, 0]
    w2f[64:128, 1] = W2[:, 0]
    v1p = np.concatenate([V1, c1.reshape(1, 32)], 0)   # [8, 32]
    v1blkF = np.zeros((128, 512), np.float32)
    for gidx in range(16):
        v1blkF[8 * gidx:8 * gidx + 8, 32 * gidx:32 * gidx + 32] = v1p
    cbf = np.zeros((128, 1154), np.float32)
    cbf[:, 0:2] = w2f
    cbf[:, 2:514] = np.tile(V2[:, 0], (128, 16))
    cbf[:, 514:1026] = v1blkF
    cbf[:, 1026:1154] = np.eye(128, dtype=np.float32)
    cf32 = np.zeros((128, 131), np.float32)
    cf32[:, 0] = 0.5 * b2[0]
    cf32[:, 1] = 0.5 * c2[0]
    cf32[:, 2:130] = np.eye(128, dtype=np.float32)
    cf32[:, 130] = 1.0
    consts = dict(
        w1d=w1k.astype(ml_dtypes.float8_e4m3),
        cbfd=cbf.astype(ml_dtypes.bfloat16),
        cf32d=cf32,
    )
    iw = np.arange(W, dtype=np.float32)

    def _prep(c):
        st, ar, en, ph, su, rel, msk = _host_prep_core(
            c, state, arch, energy, phi_local, surprise, seg_ids)
        f36 = np.concatenate([st.T, ar.T], 0)              # [36, NPAD]
        f74 = np.concatenate(
            [f36.reshape(36, NCHUNK, 2, 4096).transpose(2, 0, 1, 3).reshape(
                72, NPAD // 2),
             np.ones((1, NPAD // 2), np.float32),
             np.zeros((1, NPAD // 2), np.float32)], 0)
        featsT = np.ascontiguousarray(
            f74.reshape(2, 37, NPAD // 2).transpose(1, 0, 2)).astype(
                ml_dtypes.float8_e4m3)
        # vt12: [1(mask), a4, sur, a2_4, eph, pad]
        vt12 = np.zeros((NPAD, 12), np.float32)
        vt12[:, 0] = msk
        vt12[:, 1:5] = ar
        vt12[:, 5] = su
        vt12[:, 6:10] = ar * ar
        vt12[:, 10] = en * ph
        oh = (rel[:, None] == iw[None, :]).astype(np.float32)   # [NPAD, W]
        voh = np.concatenate([_swz(vt12), _swz(oh)], axis=2)
        return dict(featsT=featsT,
                    vohd=np.ascontiguousarray(voh).astype(ml_dtypes.bfloat16),
                    **consts)

    from concurrent.futures import ThreadPoolExecutor
    with ThreadPoolExecutor(NCORES) as ex:
        in_maps = list(ex.map(_prep, range(NCORES)))
    nc = _get_program()
    res = run_bass_kernel_spmd(nc, in_maps, list(range(NCORES)))
    global LAST_RESULT
    LAST_RESULT = res
    outs = res.results
    alls = [np.asarray(outs[c]["out_all"]) for c in range(NCORES)]
    couts = [a[:, 0:128].reshape(2048, 8) for a in alls]
    impcs = [a[:, 128:144].reshape(-1) for a in alls]
    cluster_full = np.concatenate(couts, 0).astype(np.float32)
    impc = np.concatenate(impcs, 0).astype(np.float64)

    # organism-level finale on host (exact, f64)
    K = 16384
    counts = np.bincount(seg_ids, minlength=K)
    valid = counts > 0
    n_valid = max(float(valid.sum()), 1.0)
    aggregate = cluster_full[:, 0:4].astype(np.float64)
    phi_c = cluster_full[:, 4].astype(np.float64)
    coh = cluster_full[:, 5].astype(np.float64)
    iv = np.where(valid, impc, -np.inf)
    e = np.exp(iv - iv.max())
    wc = e / e.sum()
    ga = (wc[:, None] * aggregate).sum(0)
    eg = np.exp(ga - ga.max())
    global_arch = (eg / eg.sum()).astype(np.float32)
    avg_phi = (phi_c * valid).sum() / n_valid
    spec = np.argmax(aggregate, axis=1)
    present = np.zeros(4, bool)
    for a in range(4):
        present[a] = np.any(valid & (spec == a))
    unique = float(present.sum())
    phi_global = min(1.0, avg_phi * (0.5 + 0.5 * unique / 4.0))
    vert = (coh * valid).sum() / n_valid
    self_model = np.array([*global_arch, phi_global, vert], np.float32)
    return np.concatenate([cluster_full.reshape(-1), self_model]).astype(np.float32)


# revision 43
# speedup vs baseline: 1.2245x; 1.0120x over previous
"""Trainium2 Bass kernel for nn_BottomUpIntegrator (gnn_message_passing).

Sharding: cells split at cluster boundaries across 8 cores (2048 clusters
each). Per-core segmented reductions via one-hot scatter matmuls accumulating
into a single persistent PSUM bank with a core-invariant static window
schedule (W=8).

Phase A (per 8192-cell chunk, software-pipelined):
  DMA: feats fp8(e4m3) [37,2,4096] k-tiled for DoubleRow (bias folded as
       ones-row), host-packed bf16 [1|a|sur|a^2|eph] columns + one-hot window
       rows, one merged DMA.
  PE:  mm1 in fp8e4m3 DoubleRow (2 k-tiles of 37 rows, 0.5 cyc/row), mm2
       (base logits, hs-as-weights trick), scatter (vt12 host rows -> PSUM
       parts 32:42, ew rows -> parts 0:5).
  mm1 PSUM: ring of 2x[128,1536] tiles (6 banks); drains of 1536 cols
       (relu + fp32->bf16) alternate ACT/DVE into a 24-slot SBUF hsb ring,
       amortizing per-instruction access-latency overhead.
  ACT: tanh (sigmoid = .5+.5 tanh(v/2)), exp(w).
  Pool (SBUF-only): sigmoid affine, clip chain, ew*[1,a] scatter rows.
Phase B: strided transposes (batched 4-per-PSUM-tile, 4 wide copies),
  stats chain distributed across DVE/Pool/ACT, cluster MLP via one
  block-diagonal matmul (clusters stay on partitions; V1 bias via
  ones-feature, V2 dot via bf16 elementwise+reduce).
Organism-level finale (valid mask, argmax/present, softmax weights) runs on
host in f64 from per-cluster outputs + impc (host knows exact counts).
"""
import numpy as np
import ml_dtypes

import json as _json

from concourse import bass, mybir
from concourse import bass2jax as _b2j
from concourse import bass_utils as _bu
from concourse.tile import TileContext
from concourse.bass_utils import run_bass_kernel_spmd

_orig_compile = _bu.compile_bir_kernel


def _split_waits_compile(bir_json, tmpdir, neff_name="file.neff"):
    """Walrus lowers at most ONE semaphore wait per TPB instruction struct.
    Tile emits several. Hoist extras onto injected same-engine EventSemaphore
    wait instructions immediately before the owner (semantically identical:
    engines execute in program order)."""
    d = _json.loads(bir_json)
    cnt = 0
    for fn in d["functions"]:
        for blk in fn["blocks"]:
            newlist = []
            for ins in blk["instructions"]:
                si = ins.get("sync_info")
                waits = si.get("on_wait", []) if si else []
                if si and len(waits) > 1 and ins.get("opcode") not in (
                        "EventSemaphore",):
                    for w_i, w in enumerate(waits[:-1]):
                        cnt += 1
                        newlist.append({
                            "debug": ins.get("debug", 0),
                            "engine": ins["engine"],
                            "ins": [], "outs": [],
                            "name": f"{ins['name']}-wsplit{w_i}",
                            "opcode": "EventSemaphore",
                            "sync_info": {"on_update": [], "on_wait": [w]},
                        })
                    si["on_wait"] = [waits[-1]]
                newlist.append(ins)
            blk["instructions"] = newlist
    print(f"[wait-split] hoisted {cnt} extra waits")
    return _orig_compile(_json.dumps(d).encode(), tmpdir, neff_name=neff_name)


_bu.compile_bir_kernel = _split_waits_compile
_b2j.compile_bir_kernel = _split_waits_compile

F32 = mybir.dt.float32
BF16 = mybir.dt.bfloat16
FP8E4 = mybir.dt.float8e4
AF = mybir.ActivationFunctionType
OP = mybir.AluOpType
AX = mybir.AxisListType
PM = mybir.MatmulPerfMode

NCORES = 8
KLOC = 2048            # clusters per core
NPAD = 262144          # padded cells per core
CHUNK = 8192           # cells per chunk
NCHUNK = NPAD // CHUNK # 32
W = 8                  # onehot window width (clusters)
MARGIN = 2             # window start = clip(s - MARGIN, 0, 512 - W)
NTILES = NPAD // 128   # 2048 scatter tiles per core
TPB = NTILES // 4      # tiles per 512-cluster block
PADSEG = 1.0e9
NSLOT = NPAD // 2 // 512   # 256 global mm1 psum 512-col slots
HSB_SLOTS = 24             # hsb ring capacity in 512-col slots (3 chunks)

# scatter stationary rows, accumulated in one scat bank [16, 512]: one
# 15-row matmul per 64-cell column group; lhsT = voh cols 15j:15j+15 =
# [1, a4, sur, a2_4 (host) | ew, ew*a4 (device-written slots 10:15)]
# -> scat parts: 0=cnt, 1:5=sa, 5=ssur, 6:10=ssq, 10=sew, 11:15=sewa


def _window_start(S):
    s = S % TPB
    return int(np.clip(s - MARGIN, 0, 512 - W))


def build_program():
    nc = bass.Bass(trn_type="TRN2", use_seq_codegen=True)
    featsT = nc.dram_tensor("featsT", [37, 2, NPAD // 2], FP8E4,
                            kind="ExternalInput")
    vohd = nc.dram_tensor("vohd", [NCHUNK, 128, 1024 + 64 * W], BF16,
                          kind="ExternalInput")
    w1d = nc.dram_tensor("w1d", [37, 2, 128], FP8E4, kind="ExternalInput")
    cbfd = nc.dram_tensor("cbfd", [128, 1154], BF16, kind="ExternalInput")
    cf32d = nc.dram_tensor("cf32d", [128, 131], F32, kind="ExternalInput")
    out_all = nc.dram_tensor("out_all", [128, 144], F32, kind="ExternalOutput")

    with TileContext(nc) as tc:
        with (
            tc.tile_pool(name="const", bufs=1) as cp,
            tc.tile_pool(name="feats", bufs=4) as fp,
            tc.tile_pool(name="small", bufs=4) as sp,
            tc.tile_pool(name="scatv", bufs=7) as vp,
            tc.tile_pool(name="ph_b", bufs=1) as bp,
        ):
            # ---- constants ----------------------------------------------
            w1s = cp.tile([37, 2, 128], FP8E4, tag="w1s")
            nc.sync.dma_start(out=w1s[:], in_=w1d[:])
            cbf = cp.tile([128, 1154], BF16, tag="cbf")
            nc.sync.dma_start(out=cbf[:], in_=cbfd[:])
            cf32 = cp.tile([128, 131], F32, tag="cf32")
            nc.sync.dma_start(out=cf32[:], in_=cf32d[:])
            w2bs = cbf[:, 0:2]
            v2rep = cbf[:, 2:514]
            v1blkF = cbf[:, 514:1026]
            idb_c = cbf[:, 1026:1154]
            b2hs = cf32[:, 0:1]
            c2h = cf32[:, 1:2]
            ids = cf32[:, 2:130]
            zbf = cp.tile([128, 512], BF16, tag="zbf")
            nc.vector.memset(zbf[:], 0.0)
            # hsb ring: 24 x 512-col slots of relu'd mm1 output (bf16)
            hsbring = cp.tile([128, HSB_SLOTS * 512], BF16, tag="hsbring")

            # Pre-touch DMA-loaded constants on their consuming engines so no
            # later compute instruction needs a second (DMA) semaphore wait.
            scra = cp.tile([128, 2], F32, tag="scra")
            nc.scalar.activation(out=scra[:, 0:1], in_=b2hs, func=AF.Copy)
            nc.scalar.activation(out=scra[:, 1:2], in_=c2h, func=AF.Copy)

            # persistent scatter accumulator: 1 PSUM bank [48, 512]
            pps_cm = tc.tile_pool(name="scatps", bufs=1, space="PSUM")
            pps = pps_cm.__enter__()
            scat = pps.tile([16, 512], F32, tag="scat", name="scat")
            # PE touch of PE-consumed consts (rides on Ldweights; overwritten
            # by the zeroing matmul below).
            nc.tensor.matmul(out=scat[0:1, 0:1], lhsT=ids[0:1, 0:1],
                             rhs=ids[0:1, 0:1], start=True, stop=True,
                             skip_group_check=True)
            nc.tensor.matmul(out=scat[0:1, 0:2], lhsT=w1s[0:1, 0, 0:1],
                             rhs=w1s[0:1, 0, 0:2], start=True, stop=True,
                             skip_group_check=True)
            nc.tensor.matmul(out=scat[0:2, 0:2], lhsT=w2bs[0:1, :],
                             rhs=w2bs[0:1, :], start=True, stop=True,
                             skip_group_check=True)
            nc.tensor.matmul(out=scat[:], lhsT=zbf[:, 0:16], rhs=zbf[:],
                             start=True, stop=False, skip_group_check=True)
            sc = bp.tile([16, 2048], F32, tag="sc")

            # ---- phase A ------------------------------------------------
            with (
                tc.tile_pool(name="ringps", bufs=1, space="PSUM") as ppr,
                tc.tile_pool(name="mm2ps", bufs=1, space="PSUM") as pp2,
            ):
                # ring: 6 psum banks of mm1 output split over 3 tiles; a tile
                # drains as soon as filled while the other two fill/drain, so
                # no engine waits on its own tile's refill.
                RSL = [2, 2, 2]
                LAP = sum(RSL)
                rg = [ppr.tile([128, 512 * n], F32, tag=f"rg{i}", name=f"rg{i}")
                      for i, n in enumerate(RSL)]
                RTILE = []      # lap position -> (tile idx, local slot)
                for i, n in enumerate(RSL):
                    for j in range(n):
                        RTILE.append((i, j))
                dctr = [0]      # global drain counter (engine alternation)

                # software pipeline: at iteration it --
                #   DMA(it) | mm1+drain(it-1) | mm2+chain(it-2) | scatter(it-3)
                T = {}

                def dma_stage(k):
                    ft = fp.tile([37, 2, 4096], FP8E4, tag="ft")
                    nc.sync.dma_start(out=ft[:],
                                      in_=featsT[:, :, k * 4096:(k + 1) * 4096])
                    voh = vp.tile([128, 1024 + 64 * W], BF16, tag="voh")
                    nc.sync.dma_start(out=voh[:], in_=vohd[k])
                    T[k] = dict(ft=ft, vt15=voh[:, 0:960],
                                eph=voh[:, 960:1024], oh=voh[:, 1024:])

                def _drain(ti, g_first, nslots):
                    # drain first nslots of ring tile ti; g_first = global mm1
                    # slot index of the tile's first slot (fixes hsb position)
                    cols = 512 * nslots
                    src = rg[ti][:, 0:cols]
                    c0 = (g_first % HSB_SLOTS) * 512
                    dst = hsbring[:, c0:c0 + cols]
                    if dctr[0] % 2 == 0:
                        nc.scalar.activation(out=dst, in_=src, func=AF.Relu)
                    else:
                        nc.vector.tensor_scalar(out=dst, in0=src, scalar1=0.0,
                                                scalar2=None, op0=OP.max)
                    dctr[0] += 1

                def mm1_stage(k, half):
                    d = T[k]
                    for q in range(4 * half, 4 * half + 4):
                        g = 8 * k + q
                        ti, loc = RTILE[g % LAP]
                        nc.tensor.matmul(
                            out=rg[ti][:, 512 * loc:512 * (loc + 1)],
                            lhsT=w1s[:],
                            rhs=d["ft"][:, :, 512 * q:512 * (q + 1)],
                            start=True, stop=True, perf_mode=PM.DoubleRow,
                            skip_group_check=True)
                        if loc == RSL[ti] - 1:
                            _drain(ti, g - loc, RSL[ti])
                        elif g == NSLOT - 1:
                            _drain(ti, g - loc, loc + 1)  # tail flush

                P = {}   # per 2-chunk-pair state (tanh/exp batched: ACT init
                         # overhead halves)

                def mm2_stage(k):
                    p, c = k // 2, k % 2
                    if c == 0:
                        P[p] = dict(bb=pp2.tile([128, 128], F32, tag="bb", name="bb"))
                    bb = P[p]["bb"]
                    base = ((8 * k) % HSB_SLOTS) * 512
                    for tt in range(32):
                        nc.tensor.matmul(
                            out=bb[:, 64 * c + 2 * tt:64 * c + 2 * tt + 2],
                            lhsT=hsbring[:, base + 128 * tt:base + 128 * (tt + 1)],
                            rhs=w2bs[:],
                            start=(tt == 0 and c == 0),
                            stop=(tt == 31 and c == 1),
                            skip_group_check=True)

                def chain_t(p, half=None):
                    # tanh for pair p (or one chunk of it in the tail, where
                    # waiting for the pair's second mm2 would serialize)
                    bb = P[p]["bb"]
                    if half is None or half == 0:
                        th = sp.tile([128, 128], F32, tag="th", name="th")
                        # xw cols: [xv_e | xv_o | wv_e | wv_o]
                        P[p].update(th=th,
                                    xw=sp.tile([128, 256], F32, tag="xw",
                                               name="xw"))
                    th = P[p]["th"]
                    if half is None:
                        nc.scalar.activation(out=th[:], in_=bb[:],
                                             func=AF.Tanh, bias=b2hs, scale=0.5)
                    else:
                        sl = slice(64 * half, 64 * half + 64)
                        nc.scalar.activation(out=th[:, sl], in_=bb[:, sl],
                                             func=AF.Tanh, bias=b2hs, scale=0.5)

                def chain_a(k):
                    # sig(v) = .5 + .5*tanh(.5*v); x = max(sig*eph, .01)
                    # (sig*eph < 1 always); w = x*eph; vt12 col 10 = eph
                    # last chunks: drains are done; split chains across DVE
                    # and Pool so consecutive chunks' chains run in parallel
                    eng = nc.vector if k == NCHUNK - 1 else nc.gpsimd
                    p, c = k // 2, k % 2
                    th = P[p]["th"][:, 64 * c:64 * c + 64]
                    xw = P[p]["xw"]
                    xv = xw[:, 64 * c:64 * c + 64]
                    wv = xw[:, 128 + 64 * c:192 + 64 * c]
                    eph = T[k]["eph"].rearrange("p (s o) -> p s o", o=1)
                    sg = sp.tile([128, 64], F32, tag="sg", name="sg")
                    eng.tensor_scalar(out=sg[:], in0=th, scalar1=0.5,
                                      scalar2=0.5, op0=OP.mult, op1=OP.add)
                    eng.tensor_tensor(
                        out=xv.rearrange("p (s o) -> p s o", o=1),
                        in0=sg[:].rearrange("p (s o) -> p s o", o=1),
                        in1=eph, op=OP.mult)
                    eng.tensor_scalar(out=xv, in0=xv, scalar1=0.01,
                                      scalar2=None, op0=OP.max)
                    eng.tensor_tensor(
                        out=wv.rearrange("p (s o) -> p s o", o=1),
                        in0=xv.rearrange("p (s o) -> p s o", o=1),
                        in1=eph, op=OP.mult)

                def chain_e(p, half=None):
                    # exp for pair p (or one chunk in the tail)
                    if half is None or half == 0:
                        P[p]["ew"] = sp.tile([128, 128], BF16, tag="ewt",
                                             name="ewt")
                    ew = P[p]["ew"]
                    if half is None:
                        nc.scalar.activation(out=ew[:],
                                             in_=P[p]["xw"][:, 128:256],
                                             func=AF.Exp)
                    else:
                        nc.scalar.activation(
                            out=ew[:, 64 * half:64 * half + 64],
                            in_=P[p]["xw"][:, 128 + 64 * half:192 + 64 * half],
                            func=AF.Exp)

                def chain_b(k):
                    eng = nc.vector if k in (NCHUNK - 3, NCHUNK - 1) else \
                        nc.gpsimd
                    p, c = k // 2, k % 2
                    d = T[k]
                    vt15v = d["vt15"].rearrange("p (s v) -> p s v", v=15)
                    ewt = P[p]["ew"][:, 64 * c:64 * c + 64]
                    eng.tensor_tensor(out=vt15v[:, :, 10:15],
                                      in0=vt15v[:, :, 0:5],
                                      in1=ewt.to_broadcast([128, 64, 5]),
                                      op=OP.mult)
                    if c == 1:
                        del P[p]

                def scatter_stage(k):
                    # col j -> sorted tile S = 64k + 32*(j%2) + j//2
                    d = T[k]
                    vt15 = d["vt15"]
                    for j in range(64):
                        S = 64 * k + 32 * (j % 2) + (j // 2)
                        f = _window_start(S)
                        last = (k % 8 == 7 and j == 63)
                        nc.tensor.matmul(out=scat[0:15, f:f + W],
                                         lhsT=vt15[:, 15 * j:15 * j + 15],
                                         rhs=d["oh"][:, W * j:W * j + W],
                                         start=False, stop=last,
                                         skip_group_check=True)
                    if k % 8 == 7:
                        blk = k // 8
                        dst = sc[:, 512 * blk:512 * (blk + 1)]
                        if blk == 3:
                            # tail: split halves across ACT+DVE (both idle)
                            nc.vector.tensor_copy(out=dst[:, 0:256],
                                                  in_=scat[:, 0:256])
                            nc.scalar.activation(out=dst[:, 256:512],
                                                 in_=scat[:, 256:512],
                                                 func=AF.Copy)
                        elif blk % 2 == 0:
                            nc.vector.tensor_copy(out=dst, in_=scat[:])
                        else:
                            nc.scalar.activation(out=dst, in_=scat[:],
                                                 func=AF.Copy)
                    # free stale per-chunk state
                    del T[k]

                def zero_stage():
                    # re-zero scat bank; deferred one iteration past the sc
                    # copy so its WAR wait cannot head-of-line block PE's mm1s
                    nc.tensor.matmul(out=scat[:], lhsT=zbf[:, 0:16],
                                     rhs=zbf[:], start=True, stop=False,
                                     skip_group_check=True)

                # tile_wait_until slots pin the Tile scheduler to this exact
                # per-engine interleave (ACT: drain, tanh, drain, exp) --
                # left alone it queues exp right after tanh, and the Pool
                # chain latency then stalls the whole ACT stream.
                for it in range(NCHUNK + 7):
                    if it < NCHUNK:
                        with tc.tile_wait_until(it + 0.0):
                            dma_stage(it)
                    if 2 <= it <= NCHUNK + 1:
                        with tc.tile_wait_until(it + 0.1):
                            mm1_stage(it - 2, 0)
                    if 3 <= it <= NCHUNK + 2:
                        with tc.tile_wait_until(it + 0.2):
                            mm2_stage(it - 3)
                    if 4 <= it <= NCHUNK + 1 and (it - 4) % 2 == 0:
                        with tc.tile_wait_until(it + 0.3):
                            chain_t((it - 4) // 2)
                    if it in (NCHUNK + 1, NCHUNK + 2):
                        with tc.tile_wait_until(it + 0.25):
                            chain_t(NCHUNK // 2 - 1, half=it - NCHUNK - 1)
                    if 2 <= it <= NCHUNK + 1:
                        with tc.tile_wait_until(it + 0.4):
                            mm1_stage(it - 2, 1)
                    if 4 <= it <= NCHUNK + 1:
                        with tc.tile_wait_until(it + 0.45):
                            chain_a(it - 4)
                    if it in (NCHUNK + 1, NCHUNK + 2):
                        with tc.tile_wait_until(it + 0.45):
                            chain_a(it - 3)
                    if 5 <= it <= NCHUNK + 2 and (it - 5) % 2 == 0:
                        with tc.tile_wait_until(it + 0.5):
                            chain_e((it - 5) // 2)
                    if it in (NCHUNK + 1, NCHUNK + 2):
                        with tc.tile_wait_until(it + 0.5):
                            chain_e(NCHUNK // 2 - 1, half=it - NCHUNK - 1)
                    if 5 <= it <= NCHUNK + 2:
                        with tc.tile_wait_until(it + 0.55):
                            chain_b(it - 5)
                    if it == NCHUNK + 1:
                        with tc.tile_wait_until(it + 0.56):
                            chain_b(NCHUNK - 2)
                    if it == NCHUNK + 2:
                        with tc.tile_wait_until(it + 0.56):
                            chain_b(NCHUNK - 1)
                    if 7 <= it <= NCHUNK + 6 and (it - 7) % 8 == 7 and \
                            (it - 7) // 8 < 3:
                        with tc.tile_wait_until(it + 0.58):
                            zero_stage()
                    if 6 <= it <= NCHUNK + 3:
                        with tc.tile_wait_until(it + 0.6):
                            scatter_stage(it - 6)
                    if it == NCHUNK + 3:
                        with tc.tile_wait_until(it + 0.65):
                            scatter_stage(NCHUNK - 2)
                        with tc.tile_wait_until(it + 0.7):
                            scatter_stage(NCHUNK - 1)

            # ---- phase B ------------------------------------------------
            # sc rows: 0=cnt, 1:5=sa, 5=ssur, 6:10=ssq, 10=sew, 11:15=sewa
            pps_cm.__exit__(None, None, None)

            with (
                tc.tile_pool(name="ptps", bufs=4, space="PSUM") as ppt,
                tc.tile_pool(name="ptcs", bufs=1, space="PSUM") as pptc,
                tc.tile_pool(name="mmbps", bufs=1, space="PSUM") as ppm,
            ):
                # batched transposes: 4 x [16,128] per PSUM tile, 1 wide copy
                tt = bp.tile([128, 16 * 16], F32, tag="tt")
                scv = sc[:].rearrange("p (c g) -> p g c", g=16)
                with tc.tile_wait_until(40.1):
                    for b4 in range(4):
                        pt = ppt.tile([128, 64], F32, tag="pt", name="pt")
                        for j in range(4):
                            nc.tensor.transpose(out=pt[:, 16 * j:16 * (j + 1)],
                                                in_=scv[:, 4 * b4 + j, :],
                                                identity=ids[0:16, 0:16])
                        if b4 % 2 == 0:
                            nc.vector.tensor_copy(
                                out=tt[:, 64 * b4:64 * (b4 + 1)], in_=pt[:])
                        else:
                            nc.scalar.activation(
                                out=tt[:, 64 * b4:64 * (b4 + 1)], in_=pt[:],
                                func=AF.Copy)
                tv = tt[:].rearrange("p (b q) -> p b q", q=16)
                cnt = tv[:, :, 0:1]    # [128,16,1]
                sa = tv[:, :, 1:5]
                ssur = tv[:, :, 5:6]
                ssq = tv[:, :, 6:10]
                sew = tv[:, :, 10:11]
                sewa = tv[:, :, 11:15]

                def wt(tag):
                    return bp.tile([128, 16], F32, tag=tag, name=tag)

                def v3(t):
                    return t[:].rearrange("p (b a) -> p b a", a=1)

                def w4(tag):
                    t = bp.tile([128, 64], F32, tag=tag, name=tag)
                    return t, t[:].rearrange("p (b a) -> p b a", a=4)

                # batched reciprocals: [cntc | den | cm1] -> one reciprocal
                r3 = bp.tile([128, 48], F32, tag="r3")
                rr = bp.tile([128, 48], F32, tag="rr")
                szf = wt("szf")
                with tc.tile_wait_until(40.2):
                    nc.vector.tensor_scalar(
                        out=r3[:, 0:16].rearrange("p (b a) -> p b a", a=1),
                        in0=cnt, scalar1=1.0, scalar2=None, op0=OP.max)
                    nc.vector.tensor_scalar(
                        out=r3[:, 16:32].rearrange("p (b a) -> p b a", a=1),
                        in0=sew, scalar1=1.0, scalar2=None, op0=OP.max)
                    nc.gpsimd.tensor_scalar(
                        out=r3[:, 32:48].rearrange("p (b a) -> p b a", a=1),
                        in0=cnt, scalar1=-1.0, scalar2=1.0, op0=OP.add,
                        op1=OP.max)
                    nc.vector.reciprocal(out=rr[:], in_=r3[:])
                    nc.gpsimd.tensor_scalar(out=v3(szf), in0=cnt, scalar1=0.05,
                                            scalar2=1.0, op0=OP.mult,
                                            op1=OP.min)
                rc = rr[:, 0:16]
                rden = rr[:, 16:32]
                rcm1 = rr[:, 32:48]

                # variance chain all-DVE (critical path: no cross-engine
                # hops); softmax/aggregate interleaved; perr on Pool
                agr, agrv = w4("agr")
                es, esv = w4("es")
                ssum = wt("ssum")
                rssum = wt("rssum")
                agg, aggv = w4("agg")
                mean, meanv = w4("mean")
                m2, m2v = w4("m2")
                varn, varnv = w4("varn")
                vmr = wt("vmr")
                vm4 = wt("vm4")
                phic = wt("phic")
                coh = wt("coh")
                perr = wt("perr")
                integ = wt("integ")
                oc = bp.tile([128, 144], F32, tag="oc")
                ocv = oc[:, 0:128].rearrange("p (b q) -> p b q", q=8)
                cft = bp.tile([128, 128], BF16, tag="cft")
                cfv = cft[:].rearrange("p (b q) -> p b q", q=8)
                phic_f = ocv[:, :, 4:5]       # f32 phic (output + integ/impc)
                with tc.tile_wait_until(40.3):
                    nc.vector.tensor_tensor(out=meanv, in0=sa,
                                            in1=rc.to_broadcast([128, 16, 4]),
                                            op=OP.mult)
                    nc.vector.tensor_tensor(out=agrv, in0=sewa,
                                            in1=rden.to_broadcast([128, 16, 4]),
                                            op=OP.mult)
                    nc.scalar.activation(out=es[:], in_=agr[:], func=AF.Exp)
                    nc.vector.tensor_tensor(out=m2v, in0=meanv, in1=sa,
                                            op=OP.mult)
                    nc.vector.tensor_tensor(out=varnv, in0=ssq, in1=m2v,
                                            op=OP.subtract)
                    nc.vector.tensor_reduce(out=v3(vmr), in_=varnv, axis=AX.X,
                                            op=OP.add)
                    nc.vector.tensor_tensor(out=vm4[:], in0=vmr[:],
                                            in1=rcm1, op=OP.mult)
                    # phic = 1 - min(1, 2*vm) = relu(1 - .5*vm4)  (vm = vm4/4)
                    # written twice at source precision: bf16 into the MLP
                    # feature tile, f32 into the output tile
                    nc.scalar.activation(
                        out=cfv[:, :, 4:5],
                        in_=vm4[:].rearrange("p (b a) -> p b a", a=1),
                        func=AF.Relu, bias=1.0, scale=-0.5)
                    nc.scalar.activation(
                        out=phic_f,
                        in_=vm4[:].rearrange("p (b a) -> p b a", a=1),
                        func=AF.Relu, bias=1.0, scale=-0.5)
                    nc.vector.tensor_reduce(out=v3(ssum), in_=esv, axis=AX.X,
                                            op=OP.add)
                    nc.vector.reciprocal(out=rssum[:], in_=ssum[:])
                    nc.vector.tensor_tensor(
                        out=ocv[:, :, 0:4], in0=esv,
                        in1=rssum[:].to_broadcast([128, 16, 4]), op=OP.mult)
                    nc.gpsimd.tensor_tensor(
                        out=ocv[:, :, 6:7], in0=ssur,
                        in1=rc.rearrange("p (b a) -> p b a", a=1), op=OP.mult)
                    nc.vector.tensor_scalar(
                        out=ocv[:, :, 5:6].rearrange("p b a -> p (b a)"),
                        in0=vm4[:], scalar1=-0.25, scalar2=1.0,
                        op0=OP.mult, op1=OP.add)
                    nc.gpsimd.tensor_scalar(
                        out=v3(integ),
                        in0=ocv[:, :, 6:7], scalar1=-1.0, scalar2=1.0,
                        op0=OP.mult, op1=OP.add)
                    nc.gpsimd.tensor_tensor(out=ocv[:, :, 7:8],
                                            in0=v3(integ),
                                            in1=phic_f, op=OP.mult)

                # cluster MLP input cft [128, 16 groups x 8 feats] bf16
                with tc.tile_wait_until(40.4):
                    nc.gpsimd.memset(cfv[:, :, 7:8], 1.0)
                    nc.scalar.activation(
                        out=cfv[:, :, 6:7],
                        in_=szf[:].rearrange("p (b a) -> p b a", a=1),
                        func=AF.Copy)
                    nc.vector.tensor_copy(out=cfv[:, :, 0:4],
                                          in_=ocv[:, :, 0:4])
                    nc.vector.tensor_copy(out=cfv[:, :, 5:6],
                                          in_=ocv[:, :, 5:6])
                cfT = pptc.tile([128, 128], BF16, tag="cfT")
                cftSB = bp.tile([128, 128], BF16, tag="cftSB")
                hcp = ppm.tile([128, 512], F32, tag="hcp")
                hv2 = bp.tile([128, 512], BF16, tag="hv2")
                bb2 = wt("bb2")
                basec = wt("basec")
                with tc.tile_wait_until(40.5):
                    nc.tensor.transpose(out=cfT[:], in_=cft[:], identity=idb_c)
                    nc.vector.tensor_copy(out=cftSB[:], in_=cfT[:])
                    nc.tensor.matmul(out=hcp[:], lhsT=cftSB[:], rhs=v1blkF,
                                     start=True, stop=True,
                                     skip_group_check=True)
                    # hv2 = relu(hc) * V2 in one fused op straight from psum
                    nc.vector.scalar_tensor_tensor(
                        out=hv2[:], in0=hcp[:], scalar=0.0, in1=v2rep,
                        op0=OP.max, op1=OP.mult)
                    nc.vector.tensor_reduce(
                        out=v3(bb2),
                        in_=hv2[:].rearrange("p (b h) -> p b h", h=32),
                        axis=AX.X, op=OP.add)
                    nc.scalar.activation(out=basec[:], in_=bb2[:], func=AF.Tanh,
                                         bias=c2h, scale=0.5)

                # cluster_out cols were written in place: DMA overlaps MLP
                with tc.tile_wait_until(40.45):
                    nc.sync.dma_start(out=out_all[:, 0:128], in_=oc[:, 0:128])

                impc = oc[:, 128:144]
                with tc.tile_wait_until(40.6):
                    # impc = max(basec*phic, .01): sig affine folded in; the
                    # min-1 clip never binds (basec < 1, phic <= 1)
                    nc.vector.tensor_scalar(out=basec[:], in0=basec[:],
                                            scalar1=0.5, scalar2=0.5,
                                            op0=OP.mult, op1=OP.add)
                    nc.vector.tensor_tensor(out=v3(impc), in0=v3(basec),
                                            in1=phic_f, op=OP.mult)
                    nc.vector.tensor_scalar(out=impc, in0=impc, scalar1=0.01,
                                            scalar2=None, op0=OP.max)
                    nc.sync.dma_start(out=out_all[:, 128:144],
                                      in_=oc[:, 128:144])
    return nc


_NC_CACHE = None


def _get_program():
    global _NC_CACHE
    if _NC_CACHE is None:
        _NC_CACHE = build_program()
    return _NC_CACHE


def _host_prep_core(c, state, arch, energy, phi_local, surprise, seg_ids):
    B0 = int(np.searchsorted(seg_ids, 2048 * c))
    B1 = int(np.searchsorted(seg_ids, 2048 * (c + 1)))
    Nc = B1 - B0
    lseg = (seg_ids[B0:B1] - 2048 * c).astype(np.int64)
    idx = np.full(NPAD, -1, np.int64)
    rel = np.full(NPAD, PADSEG, np.float32)
    cur = 0
    for S in range(NTILES):
        blk = S // TPB
        f = _window_start(S)
        wlo = 512 * blk + f
        whi = wlo + W
        take = min(128, int(np.searchsorted(lseg, whi)) - cur)
        if take > 0:
            assert lseg[cur] >= wlo, f"core {c} tile {S}: behind-lag"
            sl = np.arange(cur, cur + take)
            idx[S * 128:S * 128 + take] = sl
            rel[S * 128:S * 128 + take] = (lseg[sl] - wlo).astype(np.float32)
            cur += take
    assert cur == Nc, f"core {c}: {Nc - cur} cells not scheduled"
    m = idx >= 0

    def g(x):
        out = np.zeros((NPAD,) + x.shape[1:], np.float32)
        out[m] = x[B0:B1][idx[m]]
        return out

    return g(state), g(arch), g(energy), g(phi_local), g(surprise), rel, m


def _swz(x):
    """[NPAD, Q] cell-major -> [NCHUNK, 128, 64*Q] device layout."""
    Q = x.shape[1]
    return np.ascontiguousarray(
        x.reshape(NCHUNK, 2, 32, 128, Q).transpose(0, 3, 2, 1, 4).reshape(
            NCHUNK, 128, 64 * Q))


def kernel(state, arch, energy, phi_local, surprise, seg_ids, n_clusters,
           W1, b1, W2, b2, V1, c1, V2, c2):
    state = np.asarray(state, np.float32)
    arch = np.asarray(arch, np.float32)
    energy = np.asarray(energy, np.float32)
    phi_local = np.asarray(phi_local, np.float32)
    surprise = np.asarray(surprise, np.float32)
    seg_ids = np.asarray(seg_ids)
    W1 = np.asarray(W1, np.float32); b1 = np.asarray(b1, np.float32)
    W2 = np.asarray(W2, np.float32); b2 = np.asarray(b2, np.float32)
    V1 = np.asarray(V1, np.float32); c1 = np.asarray(c1, np.float32)
    V2 = np.asarray(V2, np.float32); c2 = np.asarray(c2, np.float32)

    w1f = np.zeros((74, 128), np.float32)
    w1f[0:36, 0:64] = W1
    w1f[36:72, 64:128] = W1
    w1f[72, 0:64] = b1
    w1f[72, 64:128] = b1
    w1k = np.ascontiguousarray(
        w1f.reshape(2, 37, 128).transpose(1, 0, 2))
    w2f = np.zeros((128, 2), np.float32)
    w2f[0:64, 0] = W2[:, 0]
    w2f[64:128, 1] = W2[:, 0]
    v1p = np.concatenate([V1, c1.reshape(1, 32)], 0)   # [8, 32]
    v1blkF = np.zeros((128, 512), np.float32)
    for gidx in range(16):
        v1blkF[8 * gidx:8 * gidx + 8, 32 * gidx:32 * gidx + 32] = v1p
    cbf = np.zeros((128, 1154), np.float32)
    cbf[:, 0:2] = w2f
    cbf[:, 2:514] = np.tile(V2[:, 0], (128, 16))
    cbf[:, 514:1026] = v1blkF
    cbf[:, 1026:1154] = np.eye(128, dtype=np.float32)
    cf32 = np.zeros((128, 131), np.float32)
    cf32[:, 0] = 0.5 * b2[0]
    cf32[:, 1] = 0.5 * c2[0]
    cf32[:, 2:130] = np.eye(128, dtype=np.float32)
    cf32[:, 130] = 1.0
    consts = dict(
        w1d=w1k.astype(ml_dtypes.float8_e4m3),
        cbfd=cbf.astype(ml_dtypes.bfloat16),
        cf32d=cf32,
    )
    iw = np.arange(W, dtype=np.float32)

    def _prep(c):
        st, ar, en, ph, su, rel, msk = _host_prep_core(
            c, state, arch, energy, phi_local, surprise, seg_ids)
        f36 = np.concatenate([st.T, ar.T], 0)              # [36, NPAD]
        f74 = np.concatenate(
            [f36.reshape(36, NCHUNK, 2, 4096).transpose(2, 0, 1, 3).reshape(
                72, NPAD // 2),
             np.ones((1, NPAD // 2), np.float32),
             np.zeros((1, NPAD // 2), np.float32)], 0)
        featsT = np.ascontiguousarray(
            f74.reshape(2, 37, NPAD // 2).transpose(1, 0, 2)).astype(
                ml_dtypes.float8_e4m3)
        # vt15: [1(mask), a4, sur, a2_4, 5 device ew-slots]; eph separate
        vt15 = np.zeros((NPAD, 15), np.float32)
        vt15[:, 0] = msk
        vt15[:, 1:5] = ar
        vt15[:, 5] = su
        vt15[:, 6:10] = ar * ar
        eph1 = (en * ph).reshape(-1, 1)
        oh = (rel[:, None] == iw[None, :]).astype(np.float32)   # [NPAD, W]
        voh = np.concatenate([_swz(vt15), _swz(eph1), _swz(oh)], axis=2)
        return dict(featsT=featsT,
                    vohd=np.ascontiguousarray(voh).astype(ml_dtypes.bfloat16),
                    **consts)

    from concurrent.futures import ThreadPoolExecutor
    with ThreadPoolExecutor(NCORES) as ex:
        in_maps = list(ex.map(_prep, range(NCORES)))
    nc = _get_program()
    res = run_bass_kernel_spmd(nc, in_maps, list(range(NCORES)))
    global LAST_RESULT
    LAST_RESULT = res
    outs = res.results
    alls = [np.asarray(outs[c]["out_all"]) for c in range(NCORES)]
    couts = [a[:, 0:128].reshape(2048, 8) for a in alls]
    impcs = [a[:, 128:144].reshape(-1) for a in alls]
    cluster_full = np.concatenate(couts, 0).astype(np.float32)
    impc = np.concatenate(impcs, 0).astype(np.float64)

    # organism-level finale on host (exact, f64)
    K = 16384
    counts = np.bincount(seg_ids, minlength=K)
    valid = counts > 0
    n_valid = max(float(valid.sum()), 1.0)
    aggregate = cluster_full[:, 0:4].astype(np.float64)
    phi_c = cluster_full[:, 4].astype(np.float64)
    coh = cluster_full[:, 5].astype(np.float64)
    iv = np.where(valid, impc, -np.inf)
    e = np.exp(iv - iv.max())
    wc = e / e.sum()
    ga = (wc[:, None] * aggregate).sum(0)
    eg = np.exp(ga - ga.max())
    global_arch = (eg / eg.sum()).astype(np.float32)
    avg_phi = (phi_c * valid).sum() / n_valid
    spec = np.argmax(aggregate, axis=1)
    present = np.zeros(4, bool)
    for a in range(4):
        present[a] = np.any(valid & (spec == a))
    unique = float(present.sum())
    phi_global = min(1.0, avg_phi * (0.5 + 0.5 * unique / 4.0))
    vert = (coh * valid).sum() / n_valid
    self_model = np.array([*global_arch, phi_global, vert], np.float32)
    return np.concatenate([cluster_full.reshape(-1), self_model]).astype(np.float32)
